# revision 31
# baseline (speedup 1.0000x reference)
"""Trainium2 Bass kernel for the DoubleIntegrator affine-recurrence scan.

Math reformulation (exact, validated against the sequential reference):

  mu:  vel_t = vel_0 + dt * S_t            with S_t = sum_{k<t} tanh(v_k)
       pos_t = pos_0 + t*dt*vel_0 + dt^2 * R_t
       R_t   = inclusive_scan(S_t - 0.5 * u_{t-1})
       -> two chained prefix scans per control channel.

  cov: A = I + N with N nilpotent (N^2 = 0), so A^k = I + k*N exactly and
       Sigma_t = C0 + C1 t + C2 t^2 + C3 t^3 (4x4 coefficient matrices from
       x0_cov, Q, N).  Expanding t = tau_p + f gives
       Sigma_t = sum_j tau_p^j * G_j(f) -- a K=4 matmul per output tile with a
       host-precomputed f-polynomial table whose columns are already in the
       final (f,ch)-interleaved HBM layout.

Sharding: T=1e6 timesteps split across 8 cores (125000 each, zero-padded to
128*977=125056).  Per core, time is partition-major: partition p holds the
slab [p*977, (p+1)*977).

Default path (KFUSED=1): ONE SPMD launch.  Within-core partition carries come
from a strict-lower-triangular ones matmul on per-partition accumulator sums;
cross-core carries go through a 128-byte AllGather of per-chunk sums, and are
applied OFF the scan critical path as output-time corrections (a constant
shift for velocity, constant+ramp for position) using host-precomputed
per-core weight vectors.  The cov path is fully independent: 31 float32r
K=4 matmuls against a host-built f-polynomial table, PSUM->SBUF copies split
across ACT/DVE by slack, chunked DMA out.  Fallback (KFUSED=0): two launches
(phase-A reduction kernel + host float64 seed combine + main kernel).
"""

import sys

import numpy as np

for _p in ("/opt/trn_rl_repo",):
    if _p not in sys.path:
        sys.path.insert(0, _p)

import concourse.bass as bass
import concourse.mybir as mybir
import concourse.tile as tile
from concourse import bacc
from concourse.bass_utils import run_bass_kernel_spmd


def _install_ntff_shim():
    """Provide antenv.axon_hooks (missing in this image) so trace=True works."""
    try:
        import antenv.axon_hooks  # noqa: F401
        return
    except ImportError:
        pass
    import types
    try:
        import trn_agent_boot.trn_boot as _tb
        hook = _tb._ntff_profile_via_ctypes("/opt/axon/libaxon_pjrt.so")
    except Exception:
        hook = None
    mod = types.ModuleType("antenv.axon_hooks")
    mod.get_axon_ntff_profile_hook = lambda: hook
    sys.modules["antenv.axon_hooks"] = mod


_install_ntff_shim()

F32 = mybir.dt.float32
I32 = mybir.dt.int32

T = 1_000_000
DT = 0.2
NCORES = 8
L = T // NCORES          # 125000 true timesteps per core
P = 128
F = 977                  # free-dim per partition
PADCHUNK = P * F         # 125056 padded timesteps per core
COVW = 16 * F            # 15632 cov columns per partition
MUW = 4 * F              # 3908 mean columns per partition
SIG_CHUNK = 512          # cov matmul/psum chunk (one PSUM bank of fp32)


def _build_phase_a() -> bass.Bass:
    """Per-partition reduction kernel: tot[p] = [sum u0, sum u1, sum j*u0, sum j*u1]
    with u = tanh(v) and j = f+1 the 1-based position within the partition."""
    nc = bacc.Bacc("TRN2", target_bir_lowering=False, debug=False,
                   num_devices=NCORES)
    v = nc.dram_tensor("v", [P, 2 * F], F32, kind="ExternalInput")
    tot = nc.dram_tensor("tot", [P, 4], F32, kind="ExternalOutput")

    HALF = F // 2  # pipeline the chain in two column-halves to overlap DMA
    with tile.TileContext(nc) as tc:
        with tc.tile_pool(name="main", bufs=1) as pool:
            vt = pool.tile([P, 2 * F], F32)
            ramp_i = pool.tile([P, F], I32)
            nc.gpsimd.iota(ramp_i[:], pattern=[[1, F]], base=1, channel_multiplier=0)
            ramp = pool.tile([P, F], F32)
            nc.vector.tensor_copy(ramp[:], ramp_i[:])

            # halves of the raw [f,c]-interleaved input
            nc.sync.dma_start(vt[:, :2 * HALF], v[:, :2 * HALF])
            nc.sync.dma_start(vt[:, 2 * HALF:], v[:, 2 * HALF:])

            tott = pool.tile([P, 4, 2], F32)   # [partition, col, half]
            v3 = vt[:].rearrange("p (f c) -> p f c", c=2)
            us = [pool.tile([P, F], F32, tag=f"u{ch}", name=f"u{ch}")
                  for ch in range(2)]
            dumps = [pool.tile([P, F], F32, tag=f"d{ch}", name=f"d{ch}")
                     for ch in range(2)]
            halves = [(0, HALF), (HALF, F)]
            for h, (f0, f1) in enumerate(halves):
                for ch in range(2):
                    nc.scalar.activation(
                        us[ch][:, f0:f1], v3[:, f0:f1, ch],
                        mybir.ActivationFunctionType.Tanh,
                        accum_out=tott[:, ch, h:h + 1],
                    )
                    eng = nc.vector if ch == 0 else nc.gpsimd
                    eng.scalar_tensor_tensor(
                        out=dumps[ch][:, f0:f1], in0=us[ch][:, f0:f1], scalar=1.0,
                        in1=ramp[:, f0:f1],
                        op0=mybir.AluOpType.mult, op1=mybir.AluOpType.mult,
                        accum_out=tott[:, 2 + ch, h:h + 1],
                    )
            # combine the two half-sums: tot[p, c] = tott[p, c, 0] + tott[p, c, 1]
            tsum = pool.tile([P, 4], F32)
            nc.vector.tensor_tensor(
                out=tsum[:], in0=tott[:, :, 0], in1=tott[:, :, 1],
                op=mybir.AluOpType.add)
            nc.sync.dma_start(tot[:], tsum[:])
    nc.compile()
    return nc


def _build_main() -> bass.Bass:
    """Main kernel: scans for the mean, K=4 matmul polynomial eval for cov."""
    nc = bacc.Bacc("TRN2", target_bir_lowering=False, debug=False,
                   num_devices=NCORES)
    F32R = mybir.dt.float32r
    v = nc.dram_tensor("v", [P, 2 * F], F32, kind="ExternalInput")
    gtab = nc.dram_tensor("gtab", [4, COVW], F32R, kind="ExternalInput")
    taup = nc.dram_tensor("taup", [4, P], F32R, kind="ExternalInput")
    init = nc.dram_tensor("init", [P, 4], F32, kind="ExternalInput")
    mu_out = nc.dram_tensor("mu_out", [P, MUW], F32, kind="ExternalOutput")
    cov_out = nc.dram_tensor("cov_out", [P, COVW], F32, kind="ExternalOutput")

    add = mybir.AluOpType.add
    mult = mybir.AluOpType.mult
    bypass = mybir.AluOpType.bypass

    with tile.TileContext(nc) as tc:
        with (
            tc.tile_pool(name="main", bufs=1) as pool,
            tc.tile_pool(name="sigstage", bufs=6) as stage_pool,
            tc.tile_pool(name="psum", bufs=6, space="PSUM") as psum_pool,
        ):
            # small gating DMAs first so the sigma pipeline starts immediately
            tp = pool.tile([4, P], F32R)
            nc.sync.dma_start(tp[:], taup[:])
            it = pool.tile([P, 4], F32)
            nc.sync.dma_start(it[:], init[:])
            gt = pool.tile([4, COVW], F32R)
            nc.sync.dma_start(gt[:], gtab[:])
            vt = pool.tile([P, 2 * F], F32)
            nc.sync.dma_start(vt[:], v[:])

            # ---- cov: one K=4 matmul per 512-column chunk, PSUM -> SBUF -> HBM
            n_chunks = (COVW + SIG_CHUNK - 1) // SIG_CHUNK
            for k in range(n_chunks):
                c0 = k * SIG_CHUNK
                w = min(SIG_CHUNK, COVW - c0)
                ps = psum_pool.tile([P, SIG_CHUNK], F32, tag="sig")
                # float32r: same fp32 bits, full-rate PE mode (4x fp32) for N>=256
                nc.tensor.matmul(ps[:, :w], tp[:], gt[:, c0:c0 + w],
                                 start=True, stop=True)
                st = stage_pool.tile([P, SIG_CHUNK], F32, tag="st")
                if k < 6:
                    # DVE is idle until v arrives; ACT is busy with the
                    # activation table load + tanh early on
                    nc.vector.tensor_copy(st[:, :w], ps[:, :w])
                else:
                    nc.scalar.copy(st[:, :w], ps[:, :w])
                nc.sync.dma_start(cov_out[:, c0:c0 + w], st[:, :w])

            # ---- mean: tanh, double scan, scaled strided writes
            v3 = vt[:].rearrange("p (f c) -> p f c", c=2)
            muv = pool.tile([P, MUW], F32)
            mu3 = muv[:].rearrange("p (f c) -> p f c", c=4)
            for ch in range(2):
                u = pool.tile([P, F], F32, tag=f"u{ch}")
                nc.scalar.activation(u[:], v3[:, :, ch],
                                     mybir.ActivationFunctionType.Tanh)
                s = pool.tile([P, F], F32, tag=f"s{ch}")
                nc.vector.tensor_tensor_scan(
                    out=s[:], data0=u[:], data1=u[:],
                    initial=it[:, ch:ch + 1], op0=add, op1=bypass)
                w_ = pool.tile([P, F], F32, tag=f"w{ch}")
                nc.vector.scalar_tensor_tensor(
                    out=w_[:], in0=u[:], scalar=-0.5, in1=s[:],
                    op0=mult, op1=add)
                r = pool.tile([P, F], F32, tag=f"r{ch}")
                nc.vector.tensor_tensor_scan(
                    out=r[:], data0=w_[:], data1=w_[:],
                    initial=it[:, 2 + ch:3 + ch], op0=add, op1=bypass)
                # pos = dt^2 * R, vel = dt * S  (seeds already folded in)
                nc.vector.tensor_scalar(
                    out=mu3[:, :, ch], in0=r[:], scalar1=DT * DT, scalar2=None,
                    op0=mult)
                nc.vector.tensor_scalar(
                    out=mu3[:, :, 2 + ch], in0=s[:], scalar1=DT, scalar2=None,
                    op0=mult)
            nc.sync.dma_start(mu_out[:], muv[:])
    nc.compile()
    return nc


def _build_fused() -> bass.Bass:
    """Single-launch kernel.

    The cross-core scan seeds arrive via a tiny AllGather, but they are kept
    OFF the scan critical path: scans run with local (within-core) partition
    carries seeded from an Ltri matmul, and the cross-core seed is applied at
    output time as a constant shift (velocity), and a constant + ramp shift
    (position):

        vel  = dt*S_loc  + [dt*seedS]
        pos  = dt^2*R_loc + [dt^2*seedR] + j * [dt^2*seedS]

    with j the 1-based global-in-chunk index ramp.  All bracketed per-core
    scalars are linear in the AllGathered per-chunk sums with host-provided
    weights (wmask) and constants (consts)."""
    nc = bacc.Bacc("TRN2", target_bir_lowering=False, debug=False,
                   num_devices=NCORES)
    F32R = mybir.dt.float32r
    v = nc.dram_tensor("v", [P, 2 * F], F32, kind="ExternalInput")
    gtab = nc.dram_tensor("gtab", [4, COVW], F32R, kind="ExternalInput")
    taup = nc.dram_tensor("taup", [4, P], F32R, kind="ExternalInput")
    ltri = nc.dram_tensor("ltri", [P, P], F32, kind="ExternalInput")
    ones = nc.dram_tensor("ones", [P, 1], F32, kind="ExternalInput")
    wmask = nc.dram_tensor("wmask", [NCORES, 4], F32, kind="ExternalInput")
    consts = nc.dram_tensor("consts", [1, 6], F32, kind="ExternalInput")
    mu_out = nc.dram_tensor("mu_out", [P, MUW], F32, kind="ExternalOutput")
    cov_out = nc.dram_tensor("cov_out", [P, COVW], F32, kind="ExternalOutput")

    add = mybir.AluOpType.add
    mult = mybir.AluOpType.mult
    bypass = mybir.AluOpType.bypass
    Copy = mybir.ActivationFunctionType.Copy

    with tile.TileContext(nc) as tc:
        with (
            tc.tile_pool(name="main", bufs=1) as pool,
            tc.tile_pool(name="sigstage", bufs=8) as stage_pool,
            tc.tile_pool(name="psum", bufs=5, space="PSUM") as psum_pool,
            tc.tile_pool(name="spsum", bufs=3, space="PSUM") as spsum_pool,
            tc.tile_pool(name="dram", bufs=2, space="DRAM") as dram_pool,
        ):
            # --- input DMAs: sigma lhsT first, then v (mean chain), the rest
            tp = pool.tile([4, P], F32R)
            nc.sync.dma_start(tp[:], taup[:])
            vt = pool.tile([P, 2 * F], F32)
            nc.sync.dma_start(vt[:], v[:])
            gt = pool.tile([4, COVW], F32R)
            nc.sync.dma_start(gt[:], gtab[:])
            lt = pool.tile([P, P], F32)
            nc.sync.dma_start(lt[:], ltri[:])
            on = pool.tile([P, 1], F32)
            nc.sync.dma_start(on[:], ones[:])
            wm = pool.tile([NCORES, 4], F32)
            nc.sync.dma_start(wm[:], wmask[:])
            cst = pool.tile([1, 6], F32)
            nc.sync.dma_start(cst[:], consts[:])

            # --- mean pre-path part 1: engines ACT/Pool get these FIRST so
            # tanh is not queued behind the sigma PSUM copies (in-order queues)
            ramp_i = pool.tile([P, F], I32)
            nc.gpsimd.iota(ramp_i[:], pattern=[[1, F]], base=1,
                           channel_multiplier=F)
            ramp = pool.tile([P, F], F32)
            nc.vector.tensor_copy(ramp[:], ramp_i[:])
            v3 = vt[:].rearrange("p (f c) -> p f c", c=2)
            tot = pool.tile([P, 4], F32)     # per-partition [U0,U1,Jg0,Jg1]
            us = [pool.tile([P, F], F32, tag=f"u{ch}", name=f"u{ch}")
                  for ch in range(2)]
            for ch in range(2):
                nc.scalar.activation(
                    us[ch][:], v3[:, :, ch], mybir.ActivationFunctionType.Tanh,
                    accum_out=tot[:, ch:ch + 1])

            # --- sigma chunk emitter (PE-order interleaved with mean matmuls)
            n_chunks = (COVW + SIG_CHUNK - 1) // SIG_CHUNK
            def sig_chunks(k0, k1):
                for k in range(k0, min(k1, n_chunks)):
                    c0 = k * SIG_CHUNK
                    w = min(SIG_CHUNK, COVW - c0)
                    ps = psum_pool.tile([P, SIG_CHUNK], F32, tag="sig",
                                        name=f"ps{k}")
                    nc.tensor.matmul(ps[:, :w], tp[:], gt[:, c0:c0 + w],
                                     start=True, stop=True)
                    st = stage_pool.tile([P, SIG_CHUNK], F32, tag="st",
                                         name=f"st{k}")
                    # drain PSUM on whichever engine has slack: DVE early
                    # (ACT busy with tanh) and late (ACT is the only copier
    # mid-kernel while DVE runs the scans)
                    if k < 6 or (k >= 13 and k % 2 == 0):
                        nc.vector.tensor_copy(st[:, :w], ps[:, :w])
                    else:
                        nc.scalar.copy(st[:, :w], ps[:, :w])
                    nc.sync.dma_start(cov_out[:, c0:c0 + w], st[:, :w])

            sig_chunks(0, 7)

            # --- mean pre-path part 2: chunk summary -> AllGather
            dump = pool.tile([P, F], F32)
            for ch in range(2):
                nc.vector.scalar_tensor_tensor(
                    out=dump[:], in0=us[ch][:], scalar=1.0, in1=ramp[:],
                    op0=mult, op1=mult, accum_out=tot[:, 2 + ch:3 + ch])
            csum_ps = spsum_pool.tile([P, 4], F32, tag="small", name="csum_t")[0:1, :]
            nc.tensor.matmul(csum_ps[:], on[:], tot[:], start=True, stop=True)
            csum_sb = pool.tile([1, 4], F32)
            nc.vector.tensor_copy(csum_sb[:], csum_ps[:])
            in_bounce = dram_pool.tile([1, 4], F32)
            out_bounce = dram_pool.tile([NCORES, 4], F32)
            nc.gpsimd.dma_start(in_bounce[:], csum_sb[:])
            nc.gpsimd.collective_compute(
                "AllGather", bypass, replica_groups=[list(range(NCORES))],
                ins=[in_bounce.opt()], outs=[out_bounce.opt()])
            g8 = pool.tile([NCORES, 4], F32)
            nc.gpsimd.dma_start(g8[:], out_bounce[:])

            # --- local scans (seeded by within-core partition carries only)
            carry1_ps = spsum_pool.tile([P, 2], F32, tag="small", name="c1_t")
            nc.tensor.matmul(carry1_ps[:], lt[:], tot[:, 0:2],
                             start=True, stop=True)
            muv = pool.tile([P, MUW], F32)
            mu3 = muv[:].rearrange("p (f c) -> p f c", c=4)
            totw = pool.tile([P, 2], F32)
            ss = [pool.tile([P, F], F32, tag=f"s{ch}", name=f"s{ch}")
                  for ch in range(2)]
            ws = [pool.tile([P, F], F32, tag=f"w{ch}", name=f"w{ch}")
                  for ch in range(2)]
            rs = [pool.tile([P, F], F32, tag=f"r{ch}", name=f"r{ch}")
                  for ch in range(2)]
            for ch in range(2):
                nc.vector.tensor_tensor_scan(
                    out=ss[ch][:], data0=us[ch][:], data1=us[ch][:],
                    initial=carry1_ps[:, ch:ch + 1], op0=add, op1=bypass)
                nc.vector.scalar_tensor_tensor(
                    out=ws[ch][:], in0=us[ch][:], scalar=-0.5, in1=ss[ch][:],
                    op0=mult, op1=add, accum_out=totw[:, ch:ch + 1])

            sig_chunks(7, 13)

            carry2_ps = spsum_pool.tile([P, 2], F32, tag="small", name="c2_t")
            nc.tensor.matmul(carry2_ps[:], lt[:], totw[:],
                             start=True, stop=True)
            for ch in range(2):
                nc.vector.tensor_tensor_scan(
                    out=rs[ch][:], data0=ws[ch][:], data1=ws[ch][:],
                    initial=carry2_ps[:, ch:ch + 1], op0=add, op1=bypass)

            sig_chunks(13, n_chunks)

            # --- seeds from the gathered table (collective-gated, PE tail)
            sA_ps = spsum_pool.tile([P, 2], F32, tag="small", name="sA_t")[0:1, :]
            nc.tensor.matmul(sA_ps[:], wm[:, 0:1], g8[:, 0:2],
                             start=True, stop=True)
            sB_ps = spsum_pool.tile([P, 2], F32, tag="small", name="sB_t")[0:1, :]
            nc.tensor.matmul(sB_ps[:], wm[:, 1:2], g8[:, 0:2],
                             start=True, stop=True)
            sR_ps = spsum_pool.tile([P, 2], F32, tag="small", name="sR_t")[0:1, :]
            nc.tensor.matmul(sR_ps[:], wm[:, 2:3], g8[:, 0:2],
                             start=True, stop=False)
            nc.tensor.matmul(sR_ps[:], wm[:, 3:4], g8[:, 2:4],
                             start=False, stop=True)
            seeds_sb = pool.tile([1, 6], F32)
            nc.vector.tensor_tensor(out=seeds_sb[:, 0:2], in0=sA_ps[:],
                                    in1=cst[:, 0:2], op=add)
            nc.vector.tensor_tensor(out=seeds_sb[:, 2:4], in0=sB_ps[:],
                                    in1=cst[:, 2:4], op=add)
            nc.vector.tensor_tensor(out=seeds_sb[:, 4:6], in0=sR_ps[:],
                                    in1=cst[:, 4:6], op=add)
            bc = pool.tile([P, 6], F32)
            nc.gpsimd.partition_broadcast(bc[:], seeds_sb[:])

            # --- outputs with cross-core corrections; quartered for DMA overlap
            tmps = [pool.tile([P, F], F32, tag=f"t{ch}", name=f"t{ch}")
                    for ch in range(2)]
            NQ = 4
            QW = (F + NQ - 1) // NQ
            for q in range(NQ):
                f0, f1 = q * QW, min((q + 1) * QW, F)
                for ch in range(2):
                    # vel = dt*S_loc + bias(sA)
                    nc.vector.tensor_scalar(
                        mu3[:, f0:f1, 2 + ch], ss[ch][:, f0:f1],
                        DT, bc[:, ch:ch + 1], op0=mult, op1=add)
                    # pos = dt^2*R_loc + bias(sR) + ramp*coeff(sB)
                    nc.vector.tensor_scalar(
                        tmps[ch][:, f0:f1], rs[ch][:, f0:f1],
                        DT * DT, bc[:, 4 + ch:5 + ch], op0=mult, op1=add)
                    nc.vector.scalar_tensor_tensor(
                        out=mu3[:, f0:f1, ch], in0=ramp[:, f0:f1],
                        scalar=bc[:, 2 + ch:3 + ch], in1=tmps[ch][:, f0:f1],
                        op0=mult, op1=add)
                nc.sync.dma_start(mu_out[:, 4 * f0:4 * f1], muv[:, 4 * f0:4 * f1])
    nc.compile()
    return nc


_CACHE: dict = {}
TRACE = False          # set True by test harness to collect NTFF profiles
LAST_RESULTS = {}      # phase results stashed here for the harness


def _get_kernels():
    if "a" not in _CACHE:
        _CACHE["a"] = _build_phase_a()
        _CACHE["m"] = _build_main()
    return _CACHE["a"], _CACHE["m"]


def _host_prep(v_sequence, x0_mean, x0_cov, A, Q):
    """Everything the host precomputes: padded/reshaped per-core v chunks and
    the cov polynomial tables (float64 -> fp32)."""
    v = np.ascontiguousarray(np.asarray(v_sequence, np.float32))
    vpad = np.zeros((NCORES, PADCHUNK, 2), np.float32)
    vpad[:, :L] = v.reshape(NCORES, L, 2)
    vchunks = [np.ascontiguousarray(vpad[c].reshape(P, 2 * F)) for c in range(NCORES)]

    # cov cubic coefficients
    S0 = np.asarray(x0_cov, np.float64)
    Qm = np.asarray(Q, np.float64)
    N = np.asarray(A, np.float64) - np.eye(4)
    M1 = N @ S0 + S0 @ N.T
    M2 = N @ S0 @ N.T
    P1 = N @ Qm + Qm @ N.T
    P2 = N @ Qm @ N.T
    C = [S0,
         Qm + M1 - P1 / 2 + P2 / 6,
         M2 + P1 / 2 - P2 / 2,
         P2 / 3]
    # G_j(f) = sum_{k>=j} binom(k,j) C_k f^(k-j), columns (f,ch)-interleaved
    f = np.arange(F, dtype=np.float64)
    binom = np.array([[1, 1, 1, 1], [0, 1, 2, 3], [0, 0, 1, 3], [0, 0, 0, 1]],
                     dtype=np.float64)
    gtab = np.zeros((4, F, 16), np.float64)
    for j in range(4):
        for k in range(j, 4):
            gtab[j] += binom[j, k] * np.power(f, k - j)[:, None] * C[k].reshape(16)[None, :]
    gtab = np.ascontiguousarray(gtab.reshape(4, COVW).astype(np.float32))

    taups = []
    for c in range(NCORES):
        tau = 1.0 + c * L + 977.0 * np.arange(P, dtype=np.float64)
        taups.append(np.ascontiguousarray(
            np.stack([tau ** j for j in range(4)]).astype(np.float32)))
    return vchunks, gtab, taups


def _fused_in_maps(v_sequence, x0_mean, x0_cov, A, Q):
    """Per-core input maps for the fused single-launch kernel."""
    vchunks, gtab, taups = _host_prep(v_sequence, x0_mean, x0_cov, A, Q)
    x0 = np.asarray(x0_mean, np.float64)
    ltri = np.ascontiguousarray(np.triu(np.ones((P, P)), 1).astype(np.float32))
    ones = np.ones((P, 1), np.float32)
    dt = float(DT)
    v0 = x0[2:4]      # initial velocity
    p0 = x0[0:2]      # initial position
    in_maps = []
    for c in range(NCORES):
        # columns: dt*maskS | dt^2*maskS | dt^2*wU | dt^2*wJ
        wm = np.zeros((NCORES, 4), np.float64)
        for cp in range(c):
            wm[cp, 0] = dt
            wm[cp, 1] = dt * dt
            wm[cp, 2] = dt * dt * (L * (c - cp) + 0.5)
            wm[cp, 3] = -dt * dt
        # consts: [vel bias | pos ramp coeff | pos bias] per (x,y)
        consts = np.array([[v0[0], v0[1],
                            dt * v0[0], dt * v0[1],
                            p0[0] + dt * c * L * v0[0],
                            p0[1] + dt * c * L * v0[1]]], np.float64)
        in_maps.append({
            "v": vchunks[c],
            "gtab": gtab,
            "taup": taups[c],
            "ltri": ltri,
            "ones": ones,
            "wmask": np.ascontiguousarray(wm.astype(np.float32)),
            "consts": np.ascontiguousarray(consts.astype(np.float32)),
        })
    return in_maps


def _host_seeds(tots, x0_mean):
    """Combine phase-A per-partition sums into per-partition scan initials.

    tots: list of NCORES arrays [128,4] = [sum u0, sum u1, sum j*u0, sum j*u1]
    Returns per-core [128,4] fp32: [initS_x, initS_y, initR_x, initR_y].
    """
    x0 = np.asarray(x0_mean, np.float64)
    seedS = x0[2:4] / DT            # running scan-1 state (u units)
    seedR = x0[0:2] / (DT * DT)     # running scan-2 state
    inits = []
    for c in range(NCORES):
        tot = np.asarray(tots[c], np.float64)
        U = tot[:, 0:2]             # per-partition sums of u
        J = tot[:, 2:4]             # per-partition sums of j*u (j = 1..F local)
        initS = np.empty((P, 2))
        initR = np.empty((P, 2))
        for p in range(P):
            initS[p] = seedS
            initR[p] = seedR
            flen = float(F if p < P - 1 else L - (P - 1) * F)  # true elements
            # sum over partition of scan-1 states = flen*seedS + sum (flen-j+1) u_j
            # (padded tail elements are zero so they contribute nothing)
            dR = flen * seedS + (flen + 1.0) * U[p] - J[p] - 0.5 * U[p]
            seedS = seedS + U[p]
            seedR = seedR + dR
        inits.append(np.ascontiguousarray(
            np.concatenate([initS, initR], axis=1).astype(np.float32)))
    return inits


def kernel(v_sequence, x0_mean, x0_cov, A, B, Q):
    v_sequence = np.asarray(v_sequence, np.float32)
    x0_mean = np.asarray(x0_mean, np.float32)
    x0_cov = np.asarray(x0_cov, np.float32)
    A = np.asarray(A, np.float32)
    B = np.asarray(B, np.float32)
    Q = np.asarray(Q, np.float32)

    core_ids = list(range(NCORES))
    import os
    if os.environ.get("KFUSED", "1") == "1":
        if "f" not in _CACHE:
            _CACHE["f"] = _build_fused()
        nc_f = _CACHE["f"]
        in_maps = _fused_in_maps(v_sequence, x0_mean, x0_cov, A, Q)
        out_m = run_bass_kernel_spmd(nc_f, in_maps, core_ids, trace=TRACE)
        res_m = out_m.results
        LAST_RESULTS.clear()
        LAST_RESULTS["m"] = out_m
    else:
        nc_a, nc_m = _get_kernels()
        vchunks, gtab, taups = _host_prep(v_sequence, x0_mean, x0_cov, A, Q)
        out_a = run_bass_kernel_spmd(
            nc_a, [{"v": vchunks[c]} for c in core_ids], core_ids, trace=TRACE)
        res_a = out_a.results
        inits = _host_seeds([r["tot"] for r in res_a], x0_mean)
        out_m = run_bass_kernel_spmd(
            nc_m,
            [{"v": vchunks[c], "gtab": gtab, "taup": taups[c], "init": inits[c]}
             for c in core_ids],
            core_ids, trace=TRACE)
        res_m = out_m.results
        LAST_RESULTS.clear()
        LAST_RESULTS["a"] = out_a
        LAST_RESULTS["m"] = out_m

    mean = np.empty((1, T + 1, 4), np.float32)
    cov = np.empty((1, T + 1, 4, 4), np.float32)
    mean[0, 0] = x0_mean
    cov[0, 0] = x0_cov
    for c in range(NCORES):
        mu = res_m[c]["mu_out"].reshape(PADCHUNK, 4)[:L]
        sg = res_m[c]["cov_out"].reshape(PADCHUNK, 16)[:L]
        mean[0, 1 + c * L:1 + (c + 1) * L] = mu
        cov[0, 1 + c * L:1 + (c + 1) * L] = sg.reshape(L, 4, 4)
    return mean, cov


# revision 38
# speedup vs baseline: 1.0511x; 1.0511x over previous
"""Trainium2 Bass kernel for the DoubleIntegrator affine-recurrence scan.

Math reformulation (exact, validated against the sequential reference):

  mu:  vel_t = vel_0 + dt * S_t            with S_t = sum_{k<t} tanh(v_k)
       pos_t = pos_0 + t*dt*vel_0 + dt^2 * R_t
       R_t   = inclusive_scan(S_t - 0.5 * u_{t-1})
       -> two chained prefix scans per control channel.

  cov: A = I + N with N nilpotent (N^2 = 0), so A^k = I + k*N exactly and
       Sigma_t = C0 + C1 t + C2 t^2 + C3 t^3 (4x4 coefficient matrices from
       x0_cov, Q, N).  Expanding t = tau_p + f gives
       Sigma_t = sum_j tau_p^j * G_j(f) -- a K=4 matmul per output tile with a
       host-precomputed f-polynomial table whose columns are already in the
       final (f,ch)-interleaved HBM layout.

Sharding: T=1e6 timesteps split across 8 cores (125000 each, zero-padded to
128*977=125056).  Per core, time is partition-major: partition p holds the
slab [p*977, (p+1)*977).

Default path (KFUSED=1): ONE SPMD launch.  Within-core partition carries come
from a strict-lower-triangular ones matmul on per-partition accumulator sums;
cross-core carries go through a 128-byte AllGather of per-chunk sums, and are
applied OFF the scan critical path as output-time corrections (a constant
shift for velocity, constant+ramp for position) using host-precomputed
per-core weight vectors.  The cov path is fully independent: 31 float32r
K=4 matmuls against a host-built f-polynomial table, PSUM->SBUF copies split
across ACT/DVE by slack, chunked DMA out.  Fallback (KFUSED=0): two launches
(phase-A reduction kernel + host float64 seed combine + main kernel).
"""

import sys

import numpy as np

for _p in ("/opt/trn_rl_repo",):
    if _p not in sys.path:
        sys.path.insert(0, _p)

import concourse.bass as bass
import concourse.mybir as mybir
import concourse.tile as tile
from concourse import bacc
from concourse.bass_utils import run_bass_kernel_spmd


def _install_ntff_shim():
    """Provide antenv.axon_hooks (missing in this image) so trace=True works."""
    try:
        import antenv.axon_hooks  # noqa: F401
        return
    except ImportError:
        pass
    import types
    try:
        import trn_agent_boot.trn_boot as _tb
        hook = _tb._ntff_profile_via_ctypes("/opt/axon/libaxon_pjrt.so")
    except Exception:
        hook = None
    mod = types.ModuleType("antenv.axon_hooks")
    mod.get_axon_ntff_profile_hook = lambda: hook
    sys.modules["antenv.axon_hooks"] = mod


_install_ntff_shim()

F32 = mybir.dt.float32
I32 = mybir.dt.int32

T = 1_000_000
DT = 0.2
NCORES = 8
L = T // NCORES          # 125000 true timesteps per core
P = 128
F = 977                  # free-dim per partition
PADCHUNK = P * F         # 125056 padded timesteps per core
COVW = 16 * F            # 15632 cov columns per partition
MUW = 4 * F              # 3908 mean columns per partition
SIG_CHUNK = 512          # cov matmul/psum chunk (one PSUM bank of fp32)


def _build_phase_a() -> bass.Bass:
    """Per-partition reduction kernel: tot[p] = [sum u0, sum u1, sum j*u0, sum j*u1]
    with u = tanh(v) and j = f+1 the 1-based position within the partition."""
    nc = bacc.Bacc("TRN2", target_bir_lowering=False, debug=False,
                   num_devices=NCORES)
    v = nc.dram_tensor("v", [P, 2 * F], F32, kind="ExternalInput")
    tot = nc.dram_tensor("tot", [P, 4], F32, kind="ExternalOutput")

    HALF = F // 2  # pipeline the chain in two column-halves to overlap DMA
    with tile.TileContext(nc) as tc:
        with tc.tile_pool(name="main", bufs=1) as pool:
            vt = pool.tile([P, 2 * F], F32)
            ramp_i = pool.tile([P, F], I32)
            nc.gpsimd.iota(ramp_i[:], pattern=[[1, F]], base=1, channel_multiplier=0)
            ramp = pool.tile([P, F], F32)
            nc.vector.tensor_copy(ramp[:], ramp_i[:])

            # halves of the raw [f,c]-interleaved input
            nc.sync.dma_start(vt[:, :2 * HALF], v[:, :2 * HALF])
            nc.sync.dma_start(vt[:, 2 * HALF:], v[:, 2 * HALF:])

            tott = pool.tile([P, 4, 2], F32)   # [partition, col, half]
            v3 = vt[:].rearrange("p (f c) -> p f c", c=2)
            us = [pool.tile([P, F], F32, tag=f"u{ch}", name=f"u{ch}")
                  for ch in range(2)]
            dumps = [pool.tile([P, F], F32, tag=f"d{ch}", name=f"d{ch}")
                     for ch in range(2)]
            halves = [(0, HALF), (HALF, F)]
            for h, (f0, f1) in enumerate(halves):
                for ch in range(2):
                    nc.scalar.activation(
                        us[ch][:, f0:f1], v3[:, f0:f1, ch],
                        mybir.ActivationFunctionType.Tanh,
                        accum_out=tott[:, ch, h:h + 1],
                    )
                    eng = nc.vector if ch == 0 else nc.gpsimd
                    eng.scalar_tensor_tensor(
                        out=dumps[ch][:, f0:f1], in0=us[ch][:, f0:f1], scalar=1.0,
                        in1=ramp[:, f0:f1],
                        op0=mybir.AluOpType.mult, op1=mybir.AluOpType.mult,
                        accum_out=tott[:, 2 + ch, h:h + 1],
                    )
            # combine the two half-sums: tot[p, c] = tott[p, c, 0] + tott[p, c, 1]
            tsum = pool.tile([P, 4], F32)
            nc.vector.tensor_tensor(
                out=tsum[:], in0=tott[:, :, 0], in1=tott[:, :, 1],
                op=mybir.AluOpType.add)
            nc.sync.dma_start(tot[:], tsum[:])
    nc.compile()
    return nc


def _build_main() -> bass.Bass:
    """Main kernel: scans for the mean, K=4 matmul polynomial eval for cov."""
    nc = bacc.Bacc("TRN2", target_bir_lowering=False, debug=False,
                   num_devices=NCORES)
    F32R = mybir.dt.float32r
    v = nc.dram_tensor("v", [P, 2 * F], F32, kind="ExternalInput")
    gtab = nc.dram_tensor("gtab", [4, COVW], F32R, kind="ExternalInput")
    taup = nc.dram_tensor("taup", [4, P], F32R, kind="ExternalInput")
    init = nc.dram_tensor("init", [P, 4], F32, kind="ExternalInput")
    mu_out = nc.dram_tensor("mu_out", [P, MUW], F32, kind="ExternalOutput")
    cov_out = nc.dram_tensor("cov_out", [P, COVW], F32, kind="ExternalOutput")

    add = mybir.AluOpType.add
    mult = mybir.AluOpType.mult
    bypass = mybir.AluOpType.bypass

    with tile.TileContext(nc) as tc:
        with (
            tc.tile_pool(name="main", bufs=1) as pool,
            tc.tile_pool(name="sigstage", bufs=6) as stage_pool,
            tc.tile_pool(name="psum", bufs=6, space="PSUM") as psum_pool,
        ):
            # small gating DMAs first so the sigma pipeline starts immediately
            tp = pool.tile([4, P], F32R)
            nc.sync.dma_start(tp[:], taup[:])
            it = pool.tile([P, 4], F32)
            nc.sync.dma_start(it[:], init[:])
            gt = pool.tile([4, COVW], F32R)
            nc.sync.dma_start(gt[:], gtab[:])
            vt = pool.tile([P, 2 * F], F32)
            nc.sync.dma_start(vt[:], v[:])

            # ---- cov: one K=4 matmul per 512-column chunk, PSUM -> SBUF -> HBM
            n_chunks = (COVW + SIG_CHUNK - 1) // SIG_CHUNK
            for k in range(n_chunks):
                c0 = k * SIG_CHUNK
                w = min(SIG_CHUNK, COVW - c0)
                ps = psum_pool.tile([P, SIG_CHUNK], F32, tag="sig")
                # float32r: same fp32 bits, full-rate PE mode (4x fp32) for N>=256
                nc.tensor.matmul(ps[:, :w], tp[:], gt[:, c0:c0 + w],
                                 start=True, stop=True)
                st = stage_pool.tile([P, SIG_CHUNK], F32, tag="st")
                if k < 6:
                    # DVE is idle until v arrives; ACT is busy with the
                    # activation table load + tanh early on
                    nc.vector.tensor_copy(st[:, :w], ps[:, :w])
                else:
                    nc.scalar.copy(st[:, :w], ps[:, :w])
                nc.sync.dma_start(cov_out[:, c0:c0 + w], st[:, :w])

            # ---- mean: tanh, double scan, scaled strided writes
            v3 = vt[:].rearrange("p (f c) -> p f c", c=2)
            muv = pool.tile([P, MUW], F32)
            mu3 = muv[:].rearrange("p (f c) -> p f c", c=4)
            for ch in range(2):
                u = pool.tile([P, F], F32, tag=f"u{ch}")
                nc.scalar.activation(u[:], v3[:, :, ch],
                                     mybir.ActivationFunctionType.Tanh)
                s = pool.tile([P, F], F32, tag=f"s{ch}")
                nc.vector.tensor_tensor_scan(
                    out=s[:], data0=u[:], data1=u[:],
                    initial=it[:, ch:ch + 1], op0=add, op1=bypass)
                w_ = pool.tile([P, F], F32, tag=f"w{ch}")
                nc.vector.scalar_tensor_tensor(
                    out=w_[:], in0=u[:], scalar=-0.5, in1=s[:],
                    op0=mult, op1=add)
                r = pool.tile([P, F], F32, tag=f"r{ch}")
                nc.vector.tensor_tensor_scan(
                    out=r[:], data0=w_[:], data1=w_[:],
                    initial=it[:, 2 + ch:3 + ch], op0=add, op1=bypass)
                # pos = dt^2 * R, vel = dt * S  (seeds already folded in)
                nc.vector.tensor_scalar(
                    out=mu3[:, :, ch], in0=r[:], scalar1=DT * DT, scalar2=None,
                    op0=mult)
                nc.vector.tensor_scalar(
                    out=mu3[:, :, 2 + ch], in0=s[:], scalar1=DT, scalar2=None,
                    op0=mult)
            nc.sync.dma_start(mu_out[:], muv[:])
    nc.compile()
    return nc


def _build_fused() -> bass.Bass:
    """Single-launch kernel.

    The cross-core scan seeds arrive via a tiny AllGather, but they are kept
    OFF the scan critical path: scans run with local (within-core) partition
    carries seeded from an Ltri matmul, and the cross-core seed is applied at
    output time as a constant shift (velocity), and a constant + ramp shift
    (position):

        vel  = dt*S_loc  + [dt*seedS]
        pos  = dt^2*R_loc + [dt^2*seedR] + j * [dt^2*seedS]

    with j the 1-based global-in-chunk index ramp.  All bracketed per-core
    scalars are linear in the AllGathered per-chunk sums with host-provided
    weights (wmask) and constants (consts)."""
    nc = bacc.Bacc("TRN2", target_bir_lowering=False, debug=False,
                   num_devices=NCORES)
    F32R = mybir.dt.float32r
    v = nc.dram_tensor("v", [P, 2 * F], F32, kind="ExternalInput")
    gtab = nc.dram_tensor("gtab", [4, COVW], F32R, kind="ExternalInput")
    taup = nc.dram_tensor("taup", [4, P], F32R, kind="ExternalInput")
    ltri = nc.dram_tensor("ltri", [P, P], F32, kind="ExternalInput")
    ones = nc.dram_tensor("ones", [P, 1], F32, kind="ExternalInput")
    wmask = nc.dram_tensor("wmask", [NCORES, 4], F32, kind="ExternalInput")
    consts = nc.dram_tensor("consts", [1, 6], F32, kind="ExternalInput")
    mu_out = nc.dram_tensor("mu_out", [P, MUW], F32, kind="ExternalOutput")
    cov_out = nc.dram_tensor("cov_out", [P, COVW], F32, kind="ExternalOutput")

    add = mybir.AluOpType.add
    mult = mybir.AluOpType.mult
    bypass = mybir.AluOpType.bypass
    Copy = mybir.ActivationFunctionType.Copy

    with tile.TileContext(nc) as tc:
        with (
            tc.tile_pool(name="main", bufs=1) as pool,
            tc.tile_pool(name="sigstage", bufs=8) as stage_pool,
            tc.tile_pool(name="psum", bufs=5, space="PSUM") as psum_pool,
            tc.tile_pool(name="spsum", bufs=3, space="PSUM") as spsum_pool,
            tc.tile_pool(name="dram", bufs=2, space="DRAM") as dram_pool,
        ):
            # --- input DMAs: sigma lhsT first, then v (mean chain), the rest
            tp = pool.tile([4, P], F32R)
            nc.sync.dma_start(tp[:], taup[:])
            vt = pool.tile([P, 2 * F], F32)
            nc.sync.dma_start(vt[:], v[:])
            gt = pool.tile([4, COVW], F32R)
            nc.sync.dma_start(gt[:], gtab[:])
            lt = pool.tile([P, P], F32)
            nc.sync.dma_start(lt[:], ltri[:])
            on = pool.tile([P, 1], F32)
            nc.sync.dma_start(on[:], ones[:])
            wm = pool.tile([NCORES, 4], F32)
            nc.sync.dma_start(wm[:], wmask[:])
            cst = pool.tile([1, 6], F32)
            nc.sync.dma_start(cst[:], consts[:])

            # --- mean pre-path part 1: engines ACT/Pool get these FIRST so
            # tanh is not queued behind the sigma PSUM copies (in-order queues)
            ramp_i = pool.tile([P, F], I32)
            nc.gpsimd.iota(ramp_i[:], pattern=[[1, F]], base=1,
                           channel_multiplier=F)
            ramp = pool.tile([P, F], F32)
            nc.vector.tensor_copy(ramp[:], ramp_i[:])
            v3 = vt[:].rearrange("p (f c) -> p f c", c=2)
            tot = pool.tile([P, 4], F32)     # per-partition [U0,U1,Jg0,Jg1]
            us = [pool.tile([P, F], F32, tag=f"u{ch}", name=f"u{ch}")
                  for ch in range(2)]
            for ch in range(2):
                nc.scalar.activation(
                    us[ch][:], v3[:, :, ch], mybir.ActivationFunctionType.Tanh,
                    accum_out=tot[:, ch:ch + 1])

            # --- sigma chunk emitter (PE-order interleaved with mean matmuls)
            n_chunks = (COVW + SIG_CHUNK - 1) // SIG_CHUNK
            def sig_chunks(k0, k1):
                for k in range(k0, min(k1, n_chunks)):
                    c0 = k * SIG_CHUNK
                    w = min(SIG_CHUNK, COVW - c0)
                    ps = psum_pool.tile([P, SIG_CHUNK], F32, tag="sig",
                                        name=f"ps{k}")
                    nc.tensor.matmul(ps[:, :w], tp[:], gt[:, c0:c0 + w],
                                     start=True, stop=True)
                    st = stage_pool.tile([P, SIG_CHUNK], F32, tag="st",
                                         name=f"st{k}")
                    # drain PSUM on whichever engine has slack: ACT early
                    # (DVE's queue gates the collective via dumps+csum copy),
                    # both late for drain speed
                    if k >= 13 and k % 2 == 0:
                        nc.vector.tensor_copy(st[:, :w], ps[:, :w])
                    else:
                        nc.scalar.copy(st[:, :w], ps[:, :w])
                    nc.sync.dma_start(cov_out[:, c0:c0 + w], st[:, :w])

            sig_chunks(0, 7)

            # --- mean pre-path part 2: chunk summary -> AllGather
            dump = pool.tile([P, F], F32)
            for ch in range(2):
                nc.vector.scalar_tensor_tensor(
                    out=dump[:], in0=us[ch][:], scalar=1.0, in1=ramp[:],
                    op0=mult, op1=mult, accum_out=tot[:, 2 + ch:3 + ch])
            csum_ps = spsum_pool.tile([P, 4], F32, tag="small", name="csum_t")[0:1, :]
            nc.tensor.matmul(csum_ps[:], on[:], tot[:], start=True, stop=True)
            csum_sb = pool.tile([1, 4], F32)
            nc.vector.tensor_copy(csum_sb[:], csum_ps[:])
            in_bounce = dram_pool.tile([1, 4], F32)
            out_bounce = dram_pool.tile([NCORES, 4], F32)
            nc.gpsimd.dma_start(in_bounce[:], csum_sb[:])
            nc.gpsimd.collective_compute(
                "AllGather", bypass, replica_groups=[list(range(NCORES))],
                ins=[in_bounce.opt()], outs=[out_bounce.opt()])
            g8 = pool.tile([NCORES, 4], F32)
            nc.gpsimd.dma_start(g8[:], out_bounce[:])

            # --- local scans (seeded by within-core partition carries only)
            carry1_ps = spsum_pool.tile([P, 2], F32, tag="small", name="c1_t")
            nc.tensor.matmul(carry1_ps[:], lt[:], tot[:, 0:2],
                             start=True, stop=True)
            muv = pool.tile([P, MUW], F32)
            mu3 = muv[:].rearrange("p (f c) -> p f c", c=4)
            totw = pool.tile([P, 2], F32)
            ss = [pool.tile([P, F], F32, tag=f"s{ch}", name=f"s{ch}")
                  for ch in range(2)]
            ws = [pool.tile([P, F], F32, tag=f"w{ch}", name=f"w{ch}")
                  for ch in range(2)]
            rs = [pool.tile([P, F], F32, tag=f"r{ch}", name=f"r{ch}")
                  for ch in range(2)]
            for ch in range(2):
                nc.vector.tensor_tensor_scan(
                    out=ss[ch][:], data0=us[ch][:], data1=us[ch][:],
                    initial=carry1_ps[:, ch:ch + 1], op0=add, op1=bypass)
                nc.vector.scalar_tensor_tensor(
                    out=ws[ch][:], in0=us[ch][:], scalar=-0.5, in1=ss[ch][:],
                    op0=mult, op1=add, accum_out=totw[:, ch:ch + 1])

            sig_chunks(7, 13)

            carry2_ps = spsum_pool.tile([P, 2], F32, tag="small", name="c2_t")
            nc.tensor.matmul(carry2_ps[:], lt[:], totw[:],
                             start=True, stop=True)
            for ch in range(2):
                nc.vector.tensor_tensor_scan(
                    out=rs[ch][:], data0=ws[ch][:], data1=ws[ch][:],
                    initial=carry2_ps[:, ch:ch + 1], op0=add, op1=bypass)

            sig_chunks(13, n_chunks)

            # --- seeds from the gathered table (collective-gated, PE tail)
            sA_ps = spsum_pool.tile([P, 2], F32, tag="small", name="sA_t")[0:1, :]
            nc.tensor.matmul(sA_ps[:], wm[:, 0:1], g8[:, 0:2],
                             start=True, stop=True)
            sB_ps = spsum_pool.tile([P, 2], F32, tag="small", name="sB_t")[0:1, :]
            nc.tensor.matmul(sB_ps[:], wm[:, 1:2], g8[:, 0:2],
                             start=True, stop=True)
            sR_ps = spsum_pool.tile([P, 2], F32, tag="small", name="sR_t")[0:1, :]
            nc.tensor.matmul(sR_ps[:], wm[:, 2:3], g8[:, 0:2],
                             start=True, stop=False)
            nc.tensor.matmul(sR_ps[:], wm[:, 3:4], g8[:, 2:4],
                             start=False, stop=True)
            seeds_sb = pool.tile([1, 6], F32)
            nc.vector.tensor_tensor(out=seeds_sb[:, 0:2], in0=sA_ps[:],
                                    in1=cst[:, 0:2], op=add)
            nc.vector.tensor_tensor(out=seeds_sb[:, 2:4], in0=sB_ps[:],
                                    in1=cst[:, 2:4], op=add)
            nc.vector.tensor_tensor(out=seeds_sb[:, 4:6], in0=sR_ps[:],
                                    in1=cst[:, 4:6], op=add)
            bc = pool.tile([P, 6], F32)
            nc.gpsimd.partition_broadcast(bc[:], seeds_sb[:])

            # --- outputs with cross-core corrections; quartered for DMA overlap
            tmps = [pool.tile([P, F], F32, tag=f"t{ch}", name=f"t{ch}")
                    for ch in range(2)]
            NQ = 8
            QW = (F + NQ - 1) // NQ
            for q in range(NQ):
                f0, f1 = q * QW, min((q + 1) * QW, F)
                for ch in range(2):
                    # vel = dt*S_loc + bias(sA)  (GPSIMD: parallel to DVE pos chain)
                    nc.gpsimd.tensor_scalar(
                        mu3[:, f0:f1, 2 + ch], ss[ch][:, f0:f1],
                        DT, bc[:, ch:ch + 1], op0=mult, op1=add)
                    # pos = dt^2*R_loc + bias(sR) + ramp*coeff(sB)
                    nc.vector.tensor_scalar(
                        tmps[ch][:, f0:f1], rs[ch][:, f0:f1],
                        DT * DT, bc[:, 4 + ch:5 + ch], op0=mult, op1=add)
                    nc.vector.scalar_tensor_tensor(
                        out=mu3[:, f0:f1, ch], in0=ramp[:, f0:f1],
                        scalar=bc[:, 2 + ch:3 + ch], in1=tmps[ch][:, f0:f1],
                        op0=mult, op1=add)
                nc.sync.dma_start(mu_out[:, 4 * f0:4 * f1], muv[:, 4 * f0:4 * f1])
    nc.compile()
    return nc


_CACHE: dict = {}
TRACE = False          # set True by test harness to collect NTFF profiles
LAST_RESULTS = {}      # phase results stashed here for the harness


def _get_kernels():
    if "a" not in _CACHE:
        _CACHE["a"] = _build_phase_a()
        _CACHE["m"] = _build_main()
    return _CACHE["a"], _CACHE["m"]


def _host_prep(v_sequence, x0_mean, x0_cov, A, Q):
    """Everything the host precomputes: padded/reshaped per-core v chunks and
    the cov polynomial tables (float64 -> fp32)."""
    v = np.ascontiguousarray(np.asarray(v_sequence, np.float32))
    vpad = np.zeros((NCORES, PADCHUNK, 2), np.float32)
    vpad[:, :L] = v.reshape(NCORES, L, 2)
    vchunks = [np.ascontiguousarray(vpad[c].reshape(P, 2 * F)) for c in range(NCORES)]

    # cov cubic coefficients
    S0 = np.asarray(x0_cov, np.float64)
    Qm = np.asarray(Q, np.float64)
    N = np.asarray(A, np.float64) - np.eye(4)
    M1 = N @ S0 + S0 @ N.T
    M2 = N @ S0 @ N.T
    P1 = N @ Qm + Qm @ N.T
    P2 = N @ Qm @ N.T
    C = [S0,
         Qm + M1 - P1 / 2 + P2 / 6,
         M2 + P1 / 2 - P2 / 2,
         P2 / 3]
    # G_j(f) = sum_{k>=j} binom(k,j) C_k f^(k-j), columns (f,ch)-interleaved
    f = np.arange(F, dtype=np.float64)
    binom = np.array([[1, 1, 1, 1], [0, 1, 2, 3], [0, 0, 1, 3], [0, 0, 0, 1]],
                     dtype=np.float64)
    gtab = np.zeros((4, F, 16), np.float64)
    for j in range(4):
        for k in range(j, 4):
            gtab[j] += binom[j, k] * np.power(f, k - j)[:, None] * C[k].reshape(16)[None, :]
    gtab = np.ascontiguousarray(gtab.reshape(4, COVW).astype(np.float32))

    taups = []
    for c in range(NCORES):
        tau = 1.0 + c * L + 977.0 * np.arange(P, dtype=np.float64)
        taups.append(np.ascontiguousarray(
            np.stack([tau ** j for j in range(4)]).astype(np.float32)))
    return vchunks, gtab, taups


def _fused_in_maps(v_sequence, x0_mean, x0_cov, A, Q):
    """Per-core input maps for the fused single-launch kernel."""
    vchunks, gtab, taups = _host_prep(v_sequence, x0_mean, x0_cov, A, Q)
    x0 = np.asarray(x0_mean, np.float64)
    ltri = np.ascontiguousarray(np.triu(np.ones((P, P)), 1).astype(np.float32))
    ones = np.ones((P, 1), np.float32)
    dt = float(DT)
    v0 = x0[2:4]      # initial velocity
    p0 = x0[0:2]      # initial position
    in_maps = []
    for c in range(NCORES):
        # columns: dt*maskS | dt^2*maskS | dt^2*wU | dt^2*wJ
        wm = np.zeros((NCORES, 4), np.float64)
        for cp in range(c):
            wm[cp, 0] = dt
            wm[cp, 1] = dt * dt
            wm[cp, 2] = dt * dt * (L * (c - cp) + 0.5)
            wm[cp, 3] = -dt * dt
        # consts: [vel bias | pos ramp coeff | pos bias] per (x,y)
        consts = np.array([[v0[0], v0[1],
                            dt * v0[0], dt * v0[1],
                            p0[0] + dt * c * L * v0[0],
                            p0[1] + dt * c * L * v0[1]]], np.float64)
        in_maps.append({
            "v": vchunks[c],
            "gtab": gtab,
            "taup": taups[c],
            "ltri": ltri,
            "ones": ones,
            "wmask": np.ascontiguousarray(wm.astype(np.float32)),
            "consts": np.ascontiguousarray(consts.astype(np.float32)),
        })
    return in_maps


def _host_seeds(tots, x0_mean):
    """Combine phase-A per-partition sums into per-partition scan initials.

    tots: list of NCORES arrays [128,4] = [sum u0, sum u1, sum j*u0, sum j*u1]
    Returns per-core [128,4] fp32: [initS_x, initS_y, initR_x, initR_y].
    """
    x0 = np.asarray(x0_mean, np.float64)
    seedS = x0[2:4] / DT            # running scan-1 state (u units)
    seedR = x0[0:2] / (DT * DT)     # running scan-2 state
    inits = []
    for c in range(NCORES):
        tot = np.asarray(tots[c], np.float64)
        U = tot[:, 0:2]             # per-partition sums of u
        J = tot[:, 2:4]             # per-partition sums of j*u (j = 1..F local)
        initS = np.empty((P, 2))
        initR = np.empty((P, 2))
        for p in range(P):
            initS[p] = seedS
            initR[p] = seedR
            flen = float(F if p < P - 1 else L - (P - 1) * F)  # true elements
            # sum over partition of scan-1 states = flen*seedS + sum (flen-j+1) u_j
            # (padded tail elements are zero so they contribute nothing)
            dR = flen * seedS + (flen + 1.0) * U[p] - J[p] - 0.5 * U[p]
            seedS = seedS + U[p]
            seedR = seedR + dR
        inits.append(np.ascontiguousarray(
            np.concatenate([initS, initR], axis=1).astype(np.float32)))
    return inits


def kernel(v_sequence, x0_mean, x0_cov, A, B, Q):
    v_sequence = np.asarray(v_sequence, np.float32)
    x0_mean = np.asarray(x0_mean, np.float32)
    x0_cov = np.asarray(x0_cov, np.float32)
    A = np.asarray(A, np.float32)
    B = np.asarray(B, np.float32)
    Q = np.asarray(Q, np.float32)

    core_ids = list(range(NCORES))
    import os
    if os.environ.get("KFUSED", "1") == "1":
        if "f" not in _CACHE:
            _CACHE["f"] = _build_fused()
        nc_f = _CACHE["f"]
        in_maps = _fused_in_maps(v_sequence, x0_mean, x0_cov, A, Q)
        out_m = run_bass_kernel_spmd(nc_f, in_maps, core_ids, trace=TRACE)
        res_m = out_m.results
        LAST_RESULTS.clear()
        LAST_RESULTS["m"] = out_m
    else:
        nc_a, nc_m = _get_kernels()
        vchunks, gtab, taups = _host_prep(v_sequence, x0_mean, x0_cov, A, Q)
        out_a = run_bass_kernel_spmd(
            nc_a, [{"v": vchunks[c]} for c in core_ids], core_ids, trace=TRACE)
        res_a = out_a.results
        inits = _host_seeds([r["tot"] for r in res_a], x0_mean)
        out_m = run_bass_kernel_spmd(
            nc_m,
            [{"v": vchunks[c], "gtab": gtab, "taup": taups[c], "init": inits[c]}
             for c in core_ids],
            core_ids, trace=TRACE)
        res_m = out_m.results
        LAST_RESULTS.clear()
        LAST_RESULTS["a"] = out_a
        LAST_RESULTS["m"] = out_m

    mean = np.empty((1, T + 1, 4), np.float32)
    cov = np.empty((1, T + 1, 4, 4), np.float32)
    mean[0, 0] = x0_mean
    cov[0, 0] = x0_cov
    for c in range(NCORES):
        mu = res_m[c]["mu_out"].reshape(PADCHUNK, 4)[:L]
        sg = res_m[c]["cov_out"].reshape(PADCHUNK, 16)[:L]
        mean[0, 1 + c * L:1 + (c + 1) * L] = mu
        cov[0, 1 + c * L:1 + (c + 1) * L] = sg.reshape(L, 4, 4)
    return mean, cov


# revision 42
# speedup vs baseline: 1.0607x; 1.0092x over previous
"""Trainium2 Bass kernel for the DoubleIntegrator affine-recurrence scan.

Math reformulation (exact, validated against the sequential reference):

  mu:  vel_t = vel_0 + dt * S_t            with S_t = sum_{k<t} tanh(v_k)
       pos_t = pos_0 + t*dt*vel_0 + dt^2 * R_t
       R_t   = inclusive_scan(S_t - 0.5 * u_{t-1})
       -> two chained prefix scans per control channel.

  cov: A = I + N with N nilpotent (N^2 = 0), so A^k = I + k*N exactly and
       Sigma_t = C0 + C1 t + C2 t^2 + C3 t^3 (4x4 coefficient matrices from
       x0_cov, Q, N).  Expanding t = tau_p + f gives
       Sigma_t = sum_j tau_p^j * G_j(f) -- a K=4 matmul per output tile with a
       host-precomputed f-polynomial table whose columns are already in the
       final (f,ch)-interleaved HBM layout.

Sharding: T=1e6 timesteps split across 8 cores (125000 each, zero-padded to
128*977=125056).  Per core, time is partition-major: partition p holds the
slab [p*977, (p+1)*977).

Default path (KFUSED=1): ONE SPMD launch.  Within-core partition carries come
from a strict-lower-triangular ones matmul on per-partition accumulator sums;
cross-core carries go through a 128-byte AllGather of per-chunk sums, and are
applied OFF the scan critical path as output-time corrections (a constant
shift for velocity, constant+ramp for position) using host-precomputed
per-core weight vectors.  The cov path is fully independent: 31 float32r
K=4 matmuls against a host-built f-polynomial table, PSUM->SBUF copies split
across ACT/DVE by slack, chunked DMA out.  Fallback (KFUSED=0): two launches
(phase-A reduction kernel + host float64 seed combine + main kernel).
"""

import sys

import numpy as np

for _p in ("/opt/trn_rl_repo",):
    if _p not in sys.path:
        sys.path.insert(0, _p)

import concourse.bass as bass
import concourse.mybir as mybir
import concourse.tile as tile
from concourse import bacc
from concourse.bass_utils import run_bass_kernel_spmd


def _install_ntff_shim():
    """Provide antenv.axon_hooks (missing in this image) so trace=True works."""
    try:
        import antenv.axon_hooks  # noqa: F401
        return
    except ImportError:
        pass
    import types
    try:
        import trn_agent_boot.trn_boot as _tb
        hook = _tb._ntff_profile_via_ctypes("/opt/axon/libaxon_pjrt.so")
    except Exception:
        hook = None
    mod = types.ModuleType("antenv.axon_hooks")
    mod.get_axon_ntff_profile_hook = lambda: hook
    sys.modules["antenv.axon_hooks"] = mod


_install_ntff_shim()

F32 = mybir.dt.float32
I32 = mybir.dt.int32

T = 1_000_000
DT = 0.2
NCORES = 8
L = T // NCORES          # 125000 true timesteps per core
P = 128
F = 977                  # free-dim per partition
PADCHUNK = P * F         # 125056 padded timesteps per core
COVW = 16 * F            # 15632 cov columns per partition
MUW = 4 * F              # 3908 mean columns per partition
SIG_CHUNK = 512          # cov matmul/psum chunk (one PSUM bank of fp32)


def _build_phase_a() -> bass.Bass:
    """Per-partition reduction kernel: tot[p] = [sum u0, sum u1, sum j*u0, sum j*u1]
    with u = tanh(v) and j = f+1 the 1-based position within the partition."""
    nc = bacc.Bacc("TRN2", target_bir_lowering=False, debug=False,
                   num_devices=NCORES)
    v = nc.dram_tensor("v", [P, 2 * F], F32, kind="ExternalInput")
    tot = nc.dram_tensor("tot", [P, 4], F32, kind="ExternalOutput")

    HALF = F // 2  # pipeline the chain in two column-halves to overlap DMA
    with tile.TileContext(nc) as tc:
        with tc.tile_pool(name="main", bufs=1) as pool:
            vt = pool.tile([P, 2 * F], F32)
            ramp_i = pool.tile([P, F], I32)
            nc.gpsimd.iota(ramp_i[:], pattern=[[1, F]], base=1, channel_multiplier=0)
            ramp = pool.tile([P, F], F32)
            nc.vector.tensor_copy(ramp[:], ramp_i[:])

            # halves of the raw [f,c]-interleaved input
            nc.sync.dma_start(vt[:, :2 * HALF], v[:, :2 * HALF])
            nc.sync.dma_start(vt[:, 2 * HALF:], v[:, 2 * HALF:])

            tott = pool.tile([P, 4, 2], F32)   # [partition, col, half]
            v3 = vt[:].rearrange("p (f c) -> p f c", c=2)
            us = [pool.tile([P, F], F32, tag=f"u{ch}", name=f"u{ch}")
                  for ch in range(2)]
            dumps = [pool.tile([P, F], F32, tag=f"d{ch}", name=f"d{ch}")
                     for ch in range(2)]
            halves = [(0, HALF), (HALF, F)]
            for h, (f0, f1) in enumerate(halves):
                for ch in range(2):
                    nc.scalar.activation(
                        us[ch][:, f0:f1], v3[:, f0:f1, ch],
                        mybir.ActivationFunctionType.Tanh,
                        accum_out=tott[:, ch, h:h + 1],
                    )
                    eng = nc.vector if ch == 0 else nc.gpsimd
                    eng.scalar_tensor_tensor(
                        out=dumps[ch][:, f0:f1], in0=us[ch][:, f0:f1], scalar=1.0,
                        in1=ramp[:, f0:f1],
                        op0=mybir.AluOpType.mult, op1=mybir.AluOpType.mult,
                        accum_out=tott[:, 2 + ch, h:h + 1],
                    )
            # combine the two half-sums: tot[p, c] = tott[p, c, 0] + tott[p, c, 1]
            tsum = pool.tile([P, 4], F32)
            nc.vector.tensor_tensor(
                out=tsum[:], in0=tott[:, :, 0], in1=tott[:, :, 1],
                op=mybir.AluOpType.add)
            nc.sync.dma_start(tot[:], tsum[:])
    nc.compile()
    return nc


def _build_main() -> bass.Bass:
    """Main kernel: scans for the mean, K=4 matmul polynomial eval for cov."""
    nc = bacc.Bacc("TRN2", target_bir_lowering=False, debug=False,
                   num_devices=NCORES)
    F32R = mybir.dt.float32r
    v = nc.dram_tensor("v", [P, 2 * F], F32, kind="ExternalInput")
    gtab = nc.dram_tensor("gtab", [4, COVW], F32R, kind="ExternalInput")
    taup = nc.dram_tensor("taup", [4, P], F32R, kind="ExternalInput")
    init = nc.dram_tensor("init", [P, 4], F32, kind="ExternalInput")
    mu_out = nc.dram_tensor("mu_out", [P, MUW], F32, kind="ExternalOutput")
    cov_out = nc.dram_tensor("cov_out", [P, COVW], F32, kind="ExternalOutput")

    add = mybir.AluOpType.add
    mult = mybir.AluOpType.mult
    bypass = mybir.AluOpType.bypass

    with tile.TileContext(nc) as tc:
        with (
            tc.tile_pool(name="main", bufs=1) as pool,
            tc.tile_pool(name="sigstage", bufs=6) as stage_pool,
            tc.tile_pool(name="psum", bufs=6, space="PSUM") as psum_pool,
        ):
            # small gating DMAs first so the sigma pipeline starts immediately
            tp = pool.tile([4, P], F32R)
            nc.sync.dma_start(tp[:], taup[:])
            it = pool.tile([P, 4], F32)
            nc.sync.dma_start(it[:], init[:])
            gt = pool.tile([4, COVW], F32R)
            nc.sync.dma_start(gt[:], gtab[:])
            vt = pool.tile([P, 2 * F], F32)
            nc.sync.dma_start(vt[:], v[:])

            # ---- cov: one K=4 matmul per 512-column chunk, PSUM -> SBUF -> HBM
            n_chunks = (COVW + SIG_CHUNK - 1) // SIG_CHUNK
            for k in range(n_chunks):
                c0 = k * SIG_CHUNK
                w = min(SIG_CHUNK, COVW - c0)
                ps = psum_pool.tile([P, SIG_CHUNK], F32, tag="sig")
                # float32r: same fp32 bits, full-rate PE mode (4x fp32) for N>=256
                nc.tensor.matmul(ps[:, :w], tp[:], gt[:, c0:c0 + w],
                                 start=True, stop=True)
                st = stage_pool.tile([P, SIG_CHUNK], F32, tag="st")
                if k < 6:
                    # DVE is idle until v arrives; ACT is busy with the
                    # activation table load + tanh early on
                    nc.vector.tensor_copy(st[:, :w], ps[:, :w])
                else:
                    nc.scalar.copy(st[:, :w], ps[:, :w])
                nc.sync.dma_start(cov_out[:, c0:c0 + w], st[:, :w])

            # ---- mean: tanh, double scan, scaled strided writes
            v3 = vt[:].rearrange("p (f c) -> p f c", c=2)
            muv = pool.tile([P, MUW], F32)
            mu3 = muv[:].rearrange("p (f c) -> p f c", c=4)
            for ch in range(2):
                u = pool.tile([P, F], F32, tag=f"u{ch}")
                nc.scalar.activation(u[:], v3[:, :, ch],
                                     mybir.ActivationFunctionType.Tanh)
                s = pool.tile([P, F], F32, tag=f"s{ch}")
                nc.vector.tensor_tensor_scan(
                    out=s[:], data0=u[:], data1=u[:],
                    initial=it[:, ch:ch + 1], op0=add, op1=bypass)
                w_ = pool.tile([P, F], F32, tag=f"w{ch}")
                nc.vector.scalar_tensor_tensor(
                    out=w_[:], in0=u[:], scalar=-0.5, in1=s[:],
                    op0=mult, op1=add)
                r = pool.tile([P, F], F32, tag=f"r{ch}")
                nc.vector.tensor_tensor_scan(
                    out=r[:], data0=w_[:], data1=w_[:],
                    initial=it[:, 2 + ch:3 + ch], op0=add, op1=bypass)
                # pos = dt^2 * R, vel = dt * S  (seeds already folded in)
                nc.vector.tensor_scalar(
                    out=mu3[:, :, ch], in0=r[:], scalar1=DT * DT, scalar2=None,
                    op0=mult)
                nc.vector.tensor_scalar(
                    out=mu3[:, :, 2 + ch], in0=s[:], scalar1=DT, scalar2=None,
                    op0=mult)
            nc.sync.dma_start(mu_out[:], muv[:])
    nc.compile()
    return nc


def _build_fused() -> bass.Bass:
    """Single-launch kernel.

    The cross-core scan seeds arrive via a tiny AllGather, but they are kept
    OFF the scan critical path: scans run with local (within-core) partition
    carries seeded from an Ltri matmul, and the cross-core seed is applied at
    output time as a constant shift (velocity), and a constant + ramp shift
    (position):

        vel  = dt*S_loc  + [dt*seedS]
        pos  = dt^2*R_loc + [dt^2*seedR] + j * [dt^2*seedS]

    with j the 1-based global-in-chunk index ramp.  All bracketed per-core
    scalars are linear in the AllGathered per-chunk sums with host-provided
    weights (wmask) and constants (consts)."""
    nc = bacc.Bacc("TRN2", target_bir_lowering=False, debug=False,
                   num_devices=NCORES)
    F32R = mybir.dt.float32r
    v = nc.dram_tensor("v", [P, 2 * F], F32, kind="ExternalInput")
    gtab = nc.dram_tensor("gtab", [4, COVW], F32R, kind="ExternalInput")
    taup = nc.dram_tensor("taup", [4, P], F32R, kind="ExternalInput")
    ltri = nc.dram_tensor("ltri", [P, P], F32, kind="ExternalInput")
    ones = nc.dram_tensor("ones", [P, 1], F32, kind="ExternalInput")
    wmbig = nc.dram_tensor("wmbig", [NCORES + 2, 4 * P], F32, kind="ExternalInput")
    unitr = nc.dram_tensor("unitr", [2, 4], F32, kind="ExternalInput")
    mu_out = nc.dram_tensor("mu_out", [P, MUW], F32, kind="ExternalOutput")
    cov_out = nc.dram_tensor("cov_out", [P, COVW], F32, kind="ExternalOutput")

    add = mybir.AluOpType.add
    mult = mybir.AluOpType.mult
    bypass = mybir.AluOpType.bypass
    Copy = mybir.ActivationFunctionType.Copy

    with tile.TileContext(nc) as tc:
        with (
            tc.tile_pool(name="main", bufs=1) as pool,
            tc.tile_pool(name="sigstage", bufs=8) as stage_pool,
            tc.tile_pool(name="psum", bufs=5, space="PSUM") as psum_pool,
            tc.tile_pool(name="spsum", bufs=3, space="PSUM") as spsum_pool,
            tc.tile_pool(name="dram", bufs=2, space="DRAM") as dram_pool,
        ):
            # --- input DMAs: sigma lhsT first, then v (mean chain), the rest
            tp = pool.tile([4, P], F32R)
            nc.sync.dma_start(tp[:], taup[:])
            vt = pool.tile([P, 2 * F], F32)
            nc.sync.dma_start(vt[:], v[:])
            gt = pool.tile([4, COVW], F32R)
            nc.sync.dma_start(gt[:], gtab[:])
            lt = pool.tile([P, P], F32)
            nc.sync.dma_start(lt[:], ltri[:])
            on = pool.tile([P, 1], F32)
            nc.sync.dma_start(on[:], ones[:])
            wmb = pool.tile([NCORES + 2, 4 * P], F32)
            nc.sync.dma_start(wmb[:], wmbig[:])

            # --- mean pre-path part 1: engines ACT/Pool get these FIRST so
            # tanh is not queued behind the sigma PSUM copies (in-order queues)
            ramp_i = pool.tile([P, F], I32)
            nc.gpsimd.iota(ramp_i[:], pattern=[[1, F]], base=1,
                           channel_multiplier=F)
            ramp = pool.tile([P, F], F32)
            nc.vector.tensor_copy(ramp[:], ramp_i[:])
            v3 = vt[:].rearrange("p (f c) -> p f c", c=2)
            tot = pool.tile([P, 4], F32)     # per-partition [U0,U1,Jg0,Jg1]
            us = [pool.tile([P, F], F32, tag=f"u{ch}", name=f"u{ch}")
                  for ch in range(2)]
            for ch in range(2):
                nc.scalar.activation(
                    us[ch][:], v3[:, :, ch], mybir.ActivationFunctionType.Tanh,
                    accum_out=tot[:, ch:ch + 1])

            # --- sigma chunk emitter (PE-order interleaved with mean matmuls)
            n_chunks = (COVW + SIG_CHUNK - 1) // SIG_CHUNK
            def sig_chunks(k0, k1):
                for k in range(k0, min(k1, n_chunks)):
                    c0 = k * SIG_CHUNK
                    w = min(SIG_CHUNK, COVW - c0)
                    ps = psum_pool.tile([P, SIG_CHUNK], F32, tag="sig",
                                        name=f"ps{k}")
                    nc.tensor.matmul(ps[:, :w], tp[:], gt[:, c0:c0 + w],
                                     start=True, stop=True)
                    st = stage_pool.tile([P, SIG_CHUNK], F32, tag="st",
                                         name=f"st{k}")
                    # drain PSUM on whichever engine has slack: ACT early
                    # (DVE's queue gates the collective via dumps+csum copy),
                    # both late for drain speed
                    if k >= 13 and k % 2 == 0:
                        nc.vector.tensor_copy(st[:, :w], ps[:, :w])
                    else:
                        nc.scalar.copy(st[:, :w], ps[:, :w])
                    nc.sync.dma_start(cov_out[:, c0:c0 + w], st[:, :w])

            sig_chunks(0, 7)

            # --- mean pre-path part 2: chunk summary -> AllGather
            dump = pool.tile([P, F], F32)
            for ch in range(2):
                nc.vector.scalar_tensor_tensor(
                    out=dump[:], in0=us[ch][:], scalar=1.0, in1=ramp[:],
                    op0=mult, op1=mult, accum_out=tot[:, 2 + ch:3 + ch])
            csum_ps = spsum_pool.tile([P, 4], F32, tag="small", name="csum_t")[0:1, :]
            nc.tensor.matmul(csum_ps[:], on[:], tot[:], start=True, stop=True)
            csum_sb = pool.tile([1, 4], F32)
            nc.vector.tensor_copy(csum_sb[:], csum_ps[:])
            in_bounce = dram_pool.tile([1, 4], F32)
            out_bounce = dram_pool.tile([NCORES, 4], F32)
            nc.gpsimd.dma_start(in_bounce[:], csum_sb[:])
            nc.gpsimd.collective_compute(
                "AllGather", bypass, replica_groups=[list(range(NCORES))],
                ins=[in_bounce.opt()], outs=[out_bounce.opt()])
            # gathered sums land in rows 0..7; rows 8-9 are host unit rows
            # that let the seed matmuls add per-core constants via lhsT
            g8 = pool.tile([NCORES + 2, 4], F32)
            nc.sync.dma_start(g8[NCORES:, :], unitr[:])
            nc.gpsimd.dma_start(g8[:NCORES, :], out_bounce[:])

            # --- local scans (seeded by within-core partition carries only)
            carry1_ps = spsum_pool.tile([P, 2], F32, tag="small", name="c1_t")
            nc.tensor.matmul(carry1_ps[:], lt[:], tot[:, 0:2],
                             start=True, stop=True)
            muv = pool.tile([P, MUW], F32)
            mu3 = muv[:].rearrange("p (f c) -> p f c", c=4)
            totw = pool.tile([P, 2], F32)
            ss = [pool.tile([P, F], F32, tag=f"s{ch}", name=f"s{ch}")
                  for ch in range(2)]
            ws = [pool.tile([P, F], F32, tag=f"w{ch}", name=f"w{ch}")
                  for ch in range(2)]
            rs = [pool.tile([P, F], F32, tag=f"r{ch}", name=f"r{ch}")
                  for ch in range(2)]
            for ch in range(2):
                nc.vector.tensor_tensor_scan(
                    out=ss[ch][:], data0=us[ch][:], data1=us[ch][:],
                    initial=carry1_ps[:, ch:ch + 1], op0=add, op1=bypass)
                nc.vector.scalar_tensor_tensor(
                    out=ws[ch][:], in0=us[ch][:], scalar=-0.5, in1=ss[ch][:],
                    op0=mult, op1=add, accum_out=totw[:, ch:ch + 1])

            sig_chunks(7, 13)

            carry2_ps = spsum_pool.tile([P, 2], F32, tag="small", name="c2_t")
            nc.tensor.matmul(carry2_ps[:], lt[:], totw[:],
                             start=True, stop=True)
            for ch in range(2):
                nc.vector.tensor_tensor_scan(
                    out=rs[ch][:], data0=ws[ch][:], data1=ws[ch][:],
                    initial=carry2_ps[:, ch:ch + 1], op0=add, op1=bypass)

            sig_chunks(13, n_chunks)

            # --- seeds from the gathered table (collective-gated, PE tail):
            # one [128,6] PSUM in a single PE pass -- lhsT columns hold the
            # weight vectors REPLICATED per partition (the matmul IS the
            # broadcast), and lhsT rows 8-9 against the rhs unit rows add the
            # per-core constants.
            bc_ps = spsum_pool.tile([P, 6], F32, tag="small", name="bc_t")
            nc.tensor.matmul(bc_ps[:, 0:2], wmb[:, 0:P], g8[:, 0:2],
                             start=True, stop=True)
            nc.tensor.matmul(bc_ps[:, 2:4], wmb[:, P:2 * P], g8[:, 0:2],
                             start=True, stop=True)
            nc.tensor.matmul(bc_ps[:, 4:6], wmb[:, 2 * P:3 * P], g8[:, 0:2],
                             start=True, stop=False)
            nc.tensor.matmul(bc_ps[:, 4:6], wmb[:, 3 * P:4 * P], g8[:, 2:4],
                             start=False, stop=True)
            bc = pool.tile([P, 6], F32)
            nc.vector.tensor_copy(bc[:], bc_ps[:])

            # --- outputs with cross-core corrections; quartered for DMA overlap
            tmps = [pool.tile([P, F], F32, tag=f"t{ch}", name=f"t{ch}")
                    for ch in range(2)]
            NQ = 8
            QW = (F + NQ - 1) // NQ
            for q in range(NQ):
                f0, f1 = q * QW, min((q + 1) * QW, F)
                for ch in range(2):
                    # vel = dt*S_loc + bias(sA)  (GPSIMD: parallel to DVE pos chain)
                    nc.gpsimd.tensor_scalar(
                        mu3[:, f0:f1, 2 + ch], ss[ch][:, f0:f1],
                        DT, bc[:, ch:ch + 1], op0=mult, op1=add)
                    # pos = dt^2*R_loc + bias(sR) + ramp*coeff(sB)
                    nc.vector.tensor_scalar(
                        tmps[ch][:, f0:f1], rs[ch][:, f0:f1],
                        DT * DT, bc[:, 4 + ch:5 + ch], op0=mult, op1=add)
                    nc.vector.scalar_tensor_tensor(
                        out=mu3[:, f0:f1, ch], in0=ramp[:, f0:f1],
                        scalar=bc[:, 2 + ch:3 + ch], in1=tmps[ch][:, f0:f1],
                        op0=mult, op1=add)
                nc.sync.dma_start(mu_out[:, 4 * f0:4 * f1], muv[:, 4 * f0:4 * f1])
    nc.compile()
    return nc


_CACHE: dict = {}
TRACE = False          # set True by test harness to collect NTFF profiles
LAST_RESULTS = {}      # phase results stashed here for the harness


def _get_kernels():
    if "a" not in _CACHE:
        _CACHE["a"] = _build_phase_a()
        _CACHE["m"] = _build_main()
    return _CACHE["a"], _CACHE["m"]


def _host_prep(v_sequence, x0_mean, x0_cov, A, Q):
    """Everything the host precomputes: padded/reshaped per-core v chunks and
    the cov polynomial tables (float64 -> fp32)."""
    v = np.ascontiguousarray(np.asarray(v_sequence, np.float32))
    vpad = np.zeros((NCORES, PADCHUNK, 2), np.float32)
    vpad[:, :L] = v.reshape(NCORES, L, 2)
    vchunks = [np.ascontiguousarray(vpad[c].reshape(P, 2 * F)) for c in range(NCORES)]

    # cov cubic coefficients
    S0 = np.asarray(x0_cov, np.float64)
    Qm = np.asarray(Q, np.float64)
    N = np.asarray(A, np.float64) - np.eye(4)
    M1 = N @ S0 + S0 @ N.T
    M2 = N @ S0 @ N.T
    P1 = N @ Qm + Qm @ N.T
    P2 = N @ Qm @ N.T
    C = [S0,
         Qm + M1 - P1 / 2 + P2 / 6,
         M2 + P1 / 2 - P2 / 2,
         P2 / 3]
    # G_j(f) = sum_{k>=j} binom(k,j) C_k f^(k-j), columns (f,ch)-interleaved
    f = np.arange(F, dtype=np.float64)
    binom = np.array([[1, 1, 1, 1], [0, 1, 2, 3], [0, 0, 1, 3], [0, 0, 0, 1]],
                     dtype=np.float64)
    gtab = np.zeros((4, F, 16), np.float64)
    for j in range(4):
        for k in range(j, 4):
            gtab[j] += binom[j, k] * np.power(f, k - j)[:, None] * C[k].reshape(16)[None, :]
    gtab = np.ascontiguousarray(gtab.reshape(4, COVW).astype(np.float32))

    taups = []
    for c in range(NCORES):
        tau = 1.0 + c * L + 977.0 * np.arange(P, dtype=np.float64)
        taups.append(np.ascontiguousarray(
            np.stack([tau ** j for j in range(4)]).astype(np.float32)))
    return vchunks, gtab, taups


def _fused_in_maps(v_sequence, x0_mean, x0_cov, A, Q):
    """Per-core input maps for the fused single-launch kernel."""
    vchunks, gtab, taups = _host_prep(v_sequence, x0_mean, x0_cov, A, Q)
    x0 = np.asarray(x0_mean, np.float64)
    ltri = np.ascontiguousarray(np.triu(np.ones((P, P)), 1).astype(np.float32))
    ones = np.ones((P, 1), np.float32)
    dt = float(DT)
    v0 = x0[2:4]      # initial velocity
    p0 = x0[0:2]      # initial position
    unitr = np.array([[1, 0, 0, 0], [0, 1, 0, 0]], np.float32)
    in_maps = []
    for c in range(NCORES):
        # lhsT blocks [10, 128] each: weights replicated across partitions,
        # rows 8-9 = per-core constants (applied via the rhs unit rows)
        wmb = np.zeros((NCORES + 2, 4 * P), np.float64)
        for cp in range(c):
            wmb[cp, 0:P] = dt                                      # vel bias @ U
            wmb[cp, P:2 * P] = dt * dt                             # ramp coef @ U
            wmb[cp, 2 * P:3 * P] = dt * dt * (L * (c - cp) + 0.5)  # pos bias @ U
            wmb[cp, 3 * P:4 * P] = -dt * dt                        # pos bias @ J
        wmb[NCORES + 0, 0:P] = v0[0]
        wmb[NCORES + 1, 0:P] = v0[1]
        wmb[NCORES + 0, P:2 * P] = dt * v0[0]
        wmb[NCORES + 1, P:2 * P] = dt * v0[1]
        wmb[NCORES + 0, 2 * P:3 * P] = p0[0] + dt * c * L * v0[0]
        wmb[NCORES + 1, 2 * P:3 * P] = p0[1] + dt * c * L * v0[1]
        in_maps.append({
            "v": vchunks[c],
            "gtab": gtab,
            "taup": taups[c],
            "ltri": ltri,
            "ones": ones,
            "wmbig": np.ascontiguousarray(wmb.astype(np.float32)),
            "unitr": unitr,
        })
    return in_maps


def _host_seeds(tots, x0_mean):
    """Combine phase-A per-partition sums into per-partition scan initials.

    tots: list of NCORES arrays [128,4] = [sum u0, sum u1, sum j*u0, sum j*u1]
    Returns per-core [128,4] fp32: [initS_x, initS_y, initR_x, initR_y].
    """
    x0 = np.asarray(x0_mean, np.float64)
    seedS = x0[2:4] / DT            # running scan-1 state (u units)
    seedR = x0[0:2] / (DT * DT)     # running scan-2 state
    inits = []
    for c in range(NCORES):
        tot = np.asarray(tots[c], np.float64)
        U = tot[:, 0:2]             # per-partition sums of u
        J = tot[:, 2:4]             # per-partition sums of j*u (j = 1..F local)
        initS = np.empty((P, 2))
        initR = np.empty((P, 2))
        for p in range(P):
            initS[p] = seedS
            initR[p] = seedR
            flen = float(F if p < P - 1 else L - (P - 1) * F)  # true elements
            # sum over partition of scan-1 states = flen*seedS + sum (flen-j+1) u_j
            # (padded tail elements are zero so they contribute nothing)
            dR = flen * seedS + (flen + 1.0) * U[p] - J[p] - 0.5 * U[p]
            seedS = seedS + U[p]
            seedR = seedR + dR
        inits.append(np.ascontiguousarray(
            np.concatenate([initS, initR], axis=1).astype(np.float32)))
    return inits


def kernel(v_sequence, x0_mean, x0_cov, A, B, Q):
    v_sequence = np.asarray(v_sequence, np.float32)
    x0_mean = np.asarray(x0_mean, np.float32)
    x0_cov = np.asarray(x0_cov, np.float32)
    A = np.asarray(A, np.float32)
    B = np.asarray(B, np.float32)
    Q = np.asarray(Q, np.float32)

    core_ids = list(range(NCORES))
    import os
    if os.environ.get("KFUSED", "1") == "1":
        if "f" not in _CACHE:
            _CACHE["f"] = _build_fused()
        nc_f = _CACHE["f"]
        in_maps = _fused_in_maps(v_sequence, x0_mean, x0_cov, A, Q)
        out_m = run_bass_kernel_spmd(nc_f, in_maps, core_ids, trace=TRACE)
        res_m = out_m.results
        LAST_RESULTS.clear()
        LAST_RESULTS["m"] = out_m
    else:
        nc_a, nc_m = _get_kernels()
        vchunks, gtab, taups = _host_prep(v_sequence, x0_mean, x0_cov, A, Q)
        out_a = run_bass_kernel_spmd(
            nc_a, [{"v": vchunks[c]} for c in core_ids], core_ids, trace=TRACE)
        res_a = out_a.results
        inits = _host_seeds([r["tot"] for r in res_a], x0_mean)
        out_m = run_bass_kernel_spmd(
            nc_m,
            [{"v": vchunks[c], "gtab": gtab, "taup": taups[c], "init": inits[c]}
             for c in core_ids],
            core_ids, trace=TRACE)
        res_m = out_m.results
        LAST_RESULTS.clear()
        LAST_RESULTS["a"] = out_a
        LAST_RESULTS["m"] = out_m

    mean = np.empty((1, T + 1, 4), np.float32)
    cov = np.empty((1, T + 1, 4, 4), np.float32)
    mean[0, 0] = x0_mean
    cov[0, 0] = x0_cov
    for c in range(NCORES):
        mu = res_m[c]["mu_out"].reshape(PADCHUNK, 4)[:L]
        sg = res_m[c]["cov_out"].reshape(PADCHUNK, 16)[:L]
        mean[0, 1 + c * L:1 + (c + 1) * L] = mu
        cov[0, 1 + c * L:1 + (c + 1) * L] = sg.reshape(L, 4, 4)
    return mean, cov


# revision 49
# speedup vs baseline: 1.1148x; 1.0510x over previous
"""Trainium2 Bass kernel for the DoubleIntegrator affine-recurrence scan.

Math reformulation (exact, validated against the sequential reference):

  mu:  vel_t = vel_0 + dt * S_t            with S_t = sum_{k<t} tanh(v_k)
       pos_t = pos_0 + t*dt*vel_0 + dt^2 * R_t
       R_t   = inclusive_scan(S_t - 0.5 * u_{t-1})
       -> two chained prefix scans per control channel.

  cov: A = I + N with N nilpotent (N^2 = 0), so A^k = I + k*N exactly and
       Sigma_t = C0 + C1 t + C2 t^2 + C3 t^3 (4x4 coefficient matrices from
       x0_cov, Q, N).  Expanding t = tau_p + f gives
       Sigma_t = sum_j tau_p^j * G_j(f) -- a K=4 matmul per output tile with a
       host-precomputed f-polynomial table whose columns are already in the
       final (f,ch)-interleaved HBM layout.

Sharding: T=1e6 timesteps split across 8 cores (125000 each, zero-padded to
128*977=125056).  Per core, time is partition-major: partition p holds the
slab [p*977, (p+1)*977).

Default path (KFUSED=1): ONE SPMD launch.  Within-core partition carries come
from a strict-lower-triangular ones matmul on per-partition accumulator sums;
cross-core carries go through a 128-byte AllGather of per-chunk sums, and are
applied OFF the scan critical path as output-time corrections (a constant
shift for velocity, constant+ramp for position) using host-precomputed
per-core weight vectors.  The cov path is fully independent: 31 float32r
K=4 matmuls against a host-built f-polynomial table, PSUM->SBUF copies split
across ACT/DVE by slack, chunked DMA out.  Fallback (KFUSED=0): two launches
(phase-A reduction kernel + host float64 seed combine + main kernel).
"""

import sys

import numpy as np

for _p in ("/opt/trn_rl_repo",):
    if _p not in sys.path:
        sys.path.insert(0, _p)

import concourse.bass as bass
import concourse.mybir as mybir
import concourse.tile as tile
from concourse import bacc
from concourse.bass_utils import run_bass_kernel_spmd


def _install_ntff_shim():
    """Provide antenv.axon_hooks (missing in this image) so trace=True works."""
    try:
        import antenv.axon_hooks  # noqa: F401
        return
    except ImportError:
        pass
    import types
    try:
        import trn_agent_boot.trn_boot as _tb
        hook = _tb._ntff_profile_via_ctypes("/opt/axon/libaxon_pjrt.so")
    except Exception:
        hook = None
    mod = types.ModuleType("antenv.axon_hooks")
    mod.get_axon_ntff_profile_hook = lambda: hook
    sys.modules["antenv.axon_hooks"] = mod


_install_ntff_shim()

F32 = mybir.dt.float32
I32 = mybir.dt.int32

T = 1_000_000
DT = 0.2
NCORES = 8
L = T // NCORES          # 125000 true timesteps per core
P = 128
F = 977                  # free-dim per partition
PADCHUNK = P * F         # 125056 padded timesteps per core
COVW = 16 * F            # 15632 cov columns per partition
MUW = 4 * F              # 3908 mean columns per partition
SIG_CHUNK = 512          # cov matmul/psum chunk (one PSUM bank of fp32)


def _build_phase_a() -> bass.Bass:
    """Per-partition reduction kernel: tot[p] = [sum u0, sum u1, sum j*u0, sum j*u1]
    with u = tanh(v) and j = f+1 the 1-based position within the partition."""
    nc = bacc.Bacc("TRN2", target_bir_lowering=False, debug=False,
                   num_devices=NCORES)
    v = nc.dram_tensor("v", [P, 2 * F], F32, kind="ExternalInput")
    tot = nc.dram_tensor("tot", [P, 4], F32, kind="ExternalOutput")

    HALF = F // 2  # pipeline the chain in two column-halves to overlap DMA
    with tile.TileContext(nc) as tc:
        with tc.tile_pool(name="main", bufs=1) as pool:
            vt = pool.tile([P, 2 * F], F32)
            ramp_i = pool.tile([P, F], I32)
            nc.gpsimd.iota(ramp_i[:], pattern=[[1, F]], base=1, channel_multiplier=0)
            ramp = pool.tile([P, F], F32)
            nc.vector.tensor_copy(ramp[:], ramp_i[:])

            # halves of the raw [f,c]-interleaved input
            nc.sync.dma_start(vt[:, :2 * HALF], v[:, :2 * HALF])
            nc.sync.dma_start(vt[:, 2 * HALF:], v[:, 2 * HALF:])

            tott = pool.tile([P, 4, 2], F32)   # [partition, col, half]
            v3 = vt[:].rearrange("p (f c) -> p f c", c=2)
            us = [pool.tile([P, F], F32, tag=f"u{ch}", name=f"u{ch}")
                  for ch in range(2)]
            dumps = [pool.tile([P, F], F32, tag=f"d{ch}", name=f"d{ch}")
                     for ch in range(2)]
            halves = [(0, HALF), (HALF, F)]
            for h, (f0, f1) in enumerate(halves):
                for ch in range(2):
                    nc.scalar.activation(
                        us[ch][:, f0:f1], v3[:, f0:f1, ch],
                        mybir.ActivationFunctionType.Tanh,
                        accum_out=tott[:, ch, h:h + 1],
                    )
                    eng = nc.vector if ch == 0 else nc.gpsimd
                    eng.scalar_tensor_tensor(
                        out=dumps[ch][:, f0:f1], in0=us[ch][:, f0:f1], scalar=1.0,
                        in1=ramp[:, f0:f1],
                        op0=mybir.AluOpType.mult, op1=mybir.AluOpType.mult,
                        accum_out=tott[:, 2 + ch, h:h + 1],
                    )
            # combine the two half-sums: tot[p, c] = tott[p, c, 0] + tott[p, c, 1]
            tsum = pool.tile([P, 4], F32)
            nc.vector.tensor_tensor(
                out=tsum[:], in0=tott[:, :, 0], in1=tott[:, :, 1],
                op=mybir.AluOpType.add)
            nc.sync.dma_start(tot[:], tsum[:])
    nc.compile()
    return nc


def _build_main() -> bass.Bass:
    """Main kernel: scans for the mean, K=4 matmul polynomial eval for cov."""
    nc = bacc.Bacc("TRN2", target_bir_lowering=False, debug=False,
                   num_devices=NCORES)
    F32R = mybir.dt.float32r
    v = nc.dram_tensor("v", [P, 2 * F], F32, kind="ExternalInput")
    gtab = nc.dram_tensor("gtab", [4, COVW], F32R, kind="ExternalInput")
    taup = nc.dram_tensor("taup", [4, P], F32R, kind="ExternalInput")
    init = nc.dram_tensor("init", [P, 4], F32, kind="ExternalInput")
    mu_out = nc.dram_tensor("mu_out", [P, MUW], F32, kind="ExternalOutput")
    cov_out = nc.dram_tensor("cov_out", [P, COVW], F32, kind="ExternalOutput")

    add = mybir.AluOpType.add
    mult = mybir.AluOpType.mult
    bypass = mybir.AluOpType.bypass

    with tile.TileContext(nc) as tc:
        with (
            tc.tile_pool(name="main", bufs=1) as pool,
            tc.tile_pool(name="sigstage", bufs=6) as stage_pool,
            tc.tile_pool(name="psum", bufs=6, space="PSUM") as psum_pool,
        ):
            # small gating DMAs first so the sigma pipeline starts immediately
            tp = pool.tile([4, P], F32R)
            nc.sync.dma_start(tp[:], taup[:])
            it = pool.tile([P, 4], F32)
            nc.sync.dma_start(it[:], init[:])
            gt = pool.tile([4, COVW], F32R)
            nc.sync.dma_start(gt[:], gtab[:])
            vt = pool.tile([P, 2 * F], F32)
            nc.sync.dma_start(vt[:], v[:])

            # ---- cov: one K=4 matmul per 512-column chunk, PSUM -> SBUF -> HBM
            n_chunks = (COVW + SIG_CHUNK - 1) // SIG_CHUNK
            for k in range(n_chunks):
                c0 = k * SIG_CHUNK
                w = min(SIG_CHUNK, COVW - c0)
                ps = psum_pool.tile([P, SIG_CHUNK], F32, tag="sig")
                # float32r: same fp32 bits, full-rate PE mode (4x fp32) for N>=256
                nc.tensor.matmul(ps[:, :w], tp[:], gt[:, c0:c0 + w],
                                 start=True, stop=True)
                st = stage_pool.tile([P, SIG_CHUNK], F32, tag="st")
                if k < 6:
                    # DVE is idle until v arrives; ACT is busy with the
                    # activation table load + tanh early on
                    nc.vector.tensor_copy(st[:, :w], ps[:, :w])
                else:
                    nc.scalar.copy(st[:, :w], ps[:, :w])
                nc.sync.dma_start(cov_out[:, c0:c0 + w], st[:, :w])

            # ---- mean: tanh, double scan, scaled strided writes
            v3 = vt[:].rearrange("p (f c) -> p f c", c=2)
            muv = pool.tile([P, MUW], F32)
            mu3 = muv[:].rearrange("p (f c) -> p f c", c=4)
            for ch in range(2):
                u = pool.tile([P, F], F32, tag=f"u{ch}")
                nc.scalar.activation(u[:], v3[:, :, ch],
                                     mybir.ActivationFunctionType.Tanh)
                s = pool.tile([P, F], F32, tag=f"s{ch}")
                nc.vector.tensor_tensor_scan(
                    out=s[:], data0=u[:], data1=u[:],
                    initial=it[:, ch:ch + 1], op0=add, op1=bypass)
                w_ = pool.tile([P, F], F32, tag=f"w{ch}")
                nc.vector.scalar_tensor_tensor(
                    out=w_[:], in0=u[:], scalar=-0.5, in1=s[:],
                    op0=mult, op1=add)
                r = pool.tile([P, F], F32, tag=f"r{ch}")
                nc.vector.tensor_tensor_scan(
                    out=r[:], data0=w_[:], data1=w_[:],
                    initial=it[:, 2 + ch:3 + ch], op0=add, op1=bypass)
                # pos = dt^2 * R, vel = dt * S  (seeds already folded in)
                nc.vector.tensor_scalar(
                    out=mu3[:, :, ch], in0=r[:], scalar1=DT * DT, scalar2=None,
                    op0=mult)
                nc.vector.tensor_scalar(
                    out=mu3[:, :, 2 + ch], in0=s[:], scalar1=DT, scalar2=None,
                    op0=mult)
            nc.sync.dma_start(mu_out[:], muv[:])
    nc.compile()
    return nc


def _build_fused() -> bass.Bass:
    """Single-launch kernel.

    The cross-core scan seeds arrive via a tiny AllGather, but they are kept
    OFF the scan critical path: scans run with local (within-core) partition
    carries seeded from an Ltri matmul, and the cross-core seed is applied at
    output time as a constant shift (velocity), and a constant + ramp shift
    (position):

        vel  = dt*S_loc  + [dt*seedS]
        pos  = dt^2*R_loc + [dt^2*seedR] + j * [dt^2*seedS]

    with j the 1-based global-in-chunk index ramp.  All bracketed per-core
    scalars are linear in the AllGathered per-chunk sums with host-provided
    weights (wmask) and constants (consts)."""
    nc = bacc.Bacc("TRN2", target_bir_lowering=False, debug=False,
                   num_devices=NCORES)
    F32R = mybir.dt.float32r
    v = nc.dram_tensor("v", [P, 2 * F], F32, kind="ExternalInput")
    gtab = nc.dram_tensor("gtab", [4, COVW], F32R, kind="ExternalInput")
    taup = nc.dram_tensor("taup", [4, P], F32R, kind="ExternalInput")
    ltri = nc.dram_tensor("ltri", [P, P], F32, kind="ExternalInput")
    ones = nc.dram_tensor("ones", [P, 1], F32, kind="ExternalInput")
    wmbig = nc.dram_tensor("wmbig", [NCORES + 2, 4 * P], F32, kind="ExternalInput")
    unitr = nc.dram_tensor("unitr", [2, 4], F32, kind="ExternalInput")
    mu_out = nc.dram_tensor("mu_out", [P, MUW], F32, kind="ExternalOutput")
    cov_out = nc.dram_tensor("cov_out", [P, COVW], F32, kind="ExternalOutput")

    add = mybir.AluOpType.add
    mult = mybir.AluOpType.mult
    bypass = mybir.AluOpType.bypass
    Copy = mybir.ActivationFunctionType.Copy

    with tile.TileContext(nc) as tc:
        with (
            tc.tile_pool(name="main", bufs=1) as pool,
            tc.tile_pool(name="sigstage", bufs=8) as stage_pool,
            tc.tile_pool(name="psum", bufs=5, space="PSUM") as psum_pool,
            tc.tile_pool(name="spsum", bufs=3, space="PSUM") as spsum_pool,
            tc.tile_pool(name="dram", bufs=2, space="DRAM") as dram_pool,
        ):
            # --- input DMAs: sigma lhsT first, then v (mean chain), the rest
            tp = pool.tile([4, P], F32R)
            nc.sync.dma_start(tp[:], taup[:])
            vt = pool.tile([P, 2 * F], F32)
            nc.sync.dma_start(vt[:], v[:])
            gt = pool.tile([4, COVW], F32R)
            nc.sync.dma_start(gt[:], gtab[:])
            lt = pool.tile([P, P], F32)
            nc.sync.dma_start(lt[:], ltri[:])
            on = pool.tile([P, 1], F32)
            nc.sync.dma_start(on[:], ones[:])
            wmb = pool.tile([NCORES + 2, 4 * P], F32)
            nc.sync.dma_start(wmb[:], wmbig[:])

            # --- mean pre-path part 1: engines ACT/Pool get these FIRST so
            # tanh is not queued behind the sigma PSUM copies (in-order queues)
            ramp_i = pool.tile([P, F], I32)
            nc.gpsimd.iota(ramp_i[:], pattern=[[1, F]], base=1,
                           channel_multiplier=F)
            ramp = pool.tile([P, F], F32)
            nc.vector.tensor_copy(ramp[:], ramp_i[:])
            v3 = vt[:].rearrange("p (f c) -> p f c", c=2)
            tot = pool.tile([P, 4], F32)     # per-partition [U0,U1,Jg0,Jg1]
            us = [pool.tile([P, F], F32, tag=f"u{ch}", name=f"u{ch}")
                  for ch in range(2)]
            for ch in range(2):
                nc.scalar.activation(
                    us[ch][:], v3[:, :, ch], mybir.ActivationFunctionType.Tanh,
                    accum_out=tot[:, ch:ch + 1])

            # --- sigma chunk emitter (PE-order interleaved with mean matmuls)
            n_chunks = (COVW + SIG_CHUNK - 1) // SIG_CHUNK
            def sig_chunks(k0, k1):
                for k in range(k0, min(k1, n_chunks)):
                    c0 = k * SIG_CHUNK
                    w = min(SIG_CHUNK, COVW - c0)
                    ps = psum_pool.tile([P, SIG_CHUNK], F32, tag="sig",
                                        name=f"ps{k}")
                    nc.tensor.matmul(ps[:, :w], tp[:], gt[:, c0:c0 + w],
                                     start=True, stop=True)
                    st = stage_pool.tile([P, SIG_CHUNK], F32, tag="st",
                                         name=f"st{k}")
                    # drain PSUM on whichever engine has slack: ACT early
                    # (DVE's queue gates the collective via dumps+csum copy),
                    # both late for drain speed
                    if k >= 13 and k % 2 == 0:
                        nc.vector.tensor_copy(st[:, :w], ps[:, :w])
                    else:
                        nc.scalar.copy(st[:, :w], ps[:, :w])
                    nc.sync.dma_start(cov_out[:, c0:c0 + w], st[:, :w])

            sig_chunks(0, 1)

            # --- mean pre-path part 2: chunk summary -> AllGather
            dump = pool.tile([P, F], F32)
            for ch in range(2):
                nc.vector.scalar_tensor_tensor(
                    out=dump[:], in0=us[ch][:], scalar=1.0, in1=ramp[:],
                    op0=mult, op1=mult, accum_out=tot[:, 2 + ch:3 + ch])
            csum_ps = spsum_pool.tile([P, 4], F32, tag="small", name="csum_t")[0:1, :]
            nc.tensor.matmul(csum_ps[:], on[:], tot[:], start=True, stop=True)
            csum_sb = pool.tile([1, 4], F32)
            nc.vector.tensor_copy(csum_sb[:], csum_ps[:])
            in_bounce = dram_pool.tile([1, 4], F32)
            out_bounce = dram_pool.tile([NCORES, 4], F32)
            nc.gpsimd.dma_start(in_bounce[:], csum_sb[:])
            nc.gpsimd.collective_compute(
                "AllGather", bypass, replica_groups=[list(range(NCORES))],
                ins=[in_bounce.opt()], outs=[out_bounce.opt()])
            # gathered sums land in rows 0..7; rows 8-9 are host unit rows
            # that let the seed matmuls add per-core constants via lhsT
            g8 = pool.tile([NCORES + 2, 4], F32)
            nc.sync.dma_start(g8[NCORES:, :], unitr[:])
            nc.gpsimd.dma_start(g8[:NCORES, :], out_bounce[:])

            # --- local scans (seeded by within-core partition carries only)
            carry1_ps = spsum_pool.tile([P, 2], F32, tag="small", name="c1_t")
            nc.tensor.matmul(carry1_ps[:], lt[:], tot[:, 0:2],
                             start=True, stop=True)
            muv = pool.tile([P, MUW], F32)
            mu3 = muv[:].rearrange("p (f c) -> p f c", c=4)
            totw = pool.tile([P, 2], F32)
            ss = [pool.tile([P, F], F32, tag=f"s{ch}", name=f"s{ch}")
                  for ch in range(2)]
            ws = [pool.tile([P, F], F32, tag=f"w{ch}", name=f"w{ch}")
                  for ch in range(2)]
            rs = [pool.tile([P, F], F32, tag=f"r{ch}", name=f"r{ch}")
                  for ch in range(2)]
            for ch in range(2):
                nc.vector.tensor_tensor_scan(
                    out=ss[ch][:], data0=us[ch][:], data1=us[ch][:],
                    initial=carry1_ps[:, ch:ch + 1], op0=add, op1=bypass)
                nc.vector.scalar_tensor_tensor(
                    out=ws[ch][:], in0=us[ch][:], scalar=-0.5, in1=ss[ch][:],
                    op0=mult, op1=add, accum_out=totw[:, ch:ch + 1])

            sig_chunks(1, 13)

            carry2_ps = spsum_pool.tile([P, 2], F32, tag="small", name="c2_t")
            nc.tensor.matmul(carry2_ps[:], lt[:], totw[:],
                             start=True, stop=True)
            for ch in range(2):
                nc.vector.tensor_tensor_scan(
                    out=rs[ch][:], data0=ws[ch][:], data1=ws[ch][:],
                    initial=carry2_ps[:, ch:ch + 1], op0=add, op1=bypass)

            sig_chunks(13, n_chunks)

            # --- seeds from the gathered table (collective-gated, PE tail):
            # one [128,6] PSUM in a single PE pass -- lhsT columns hold the
            # weight vectors REPLICATED per partition (the matmul IS the
            # broadcast), and lhsT rows 8-9 against the rhs unit rows add the
            # per-core constants.
            bc_ps = spsum_pool.tile([P, 6], F32, tag="small", name="bc_t")
            nc.tensor.matmul(bc_ps[:, 0:2], wmb[:, 0:P], g8[:, 0:2],
                             start=True, stop=True)
            nc.tensor.matmul(bc_ps[:, 2:4], wmb[:, P:2 * P], g8[:, 0:2],
                             start=True, stop=True)
            nc.tensor.matmul(bc_ps[:, 4:6], wmb[:, 2 * P:3 * P], g8[:, 0:2],
                             start=True, stop=False)
            nc.tensor.matmul(bc_ps[:, 4:6], wmb[:, 3 * P:4 * P], g8[:, 2:4],
                             start=False, stop=True)
            bc = pool.tile([P, 6], F32)
            nc.vector.tensor_copy(bc[:], bc_ps[:])

            # --- outputs with cross-core corrections; quartered for DMA overlap
            tmps = [pool.tile([P, F], F32, tag=f"t{ch}", name=f"t{ch}")
                    for ch in range(2)]
            NQ = 8
            QW = (F + NQ - 1) // NQ
            for q in range(NQ):
                f0, f1 = q * QW, min((q + 1) * QW, F)
                for ch in range(2):
                    # vel = dt*S_loc + bias(sA)  (GPSIMD: parallel to DVE pos chain)
                    nc.gpsimd.tensor_scalar(
                        mu3[:, f0:f1, 2 + ch], ss[ch][:, f0:f1],
                        DT, bc[:, ch:ch + 1], op0=mult, op1=add)
                    # pos = dt^2*R_loc + bias(sR) + ramp*coeff(sB)
                    nc.vector.tensor_scalar(
                        tmps[ch][:, f0:f1], rs[ch][:, f0:f1],
                        DT * DT, bc[:, 4 + ch:5 + ch], op0=mult, op1=add)
                    nc.vector.scalar_tensor_tensor(
                        out=mu3[:, f0:f1, ch], in0=ramp[:, f0:f1],
                        scalar=bc[:, 2 + ch:3 + ch], in1=tmps[ch][:, f0:f1],
                        op0=mult, op1=add)
                nc.sync.dma_start(mu_out[:, 4 * f0:4 * f1], muv[:, 4 * f0:4 * f1])
    nc.compile()
    return nc


_CACHE: dict = {}
TRACE = False          # set True by test harness to collect NTFF profiles
LAST_RESULTS = {}      # phase results stashed here for the harness


def _get_kernels():
    if "a" not in _CACHE:
        _CACHE["a"] = _build_phase_a()
        _CACHE["m"] = _build_main()
    return _CACHE["a"], _CACHE["m"]


def _host_prep(v_sequence, x0_mean, x0_cov, A, Q):
    """Everything the host precomputes: padded/reshaped per-core v chunks and
    the cov polynomial tables (float64 -> fp32)."""
    v = np.ascontiguousarray(np.asarray(v_sequence, np.float32))
    vpad = np.zeros((NCORES, PADCHUNK, 2), np.float32)
    vpad[:, :L] = v.reshape(NCORES, L, 2)
    vchunks = [np.ascontiguousarray(vpad[c].reshape(P, 2 * F)) for c in range(NCORES)]

    # cov cubic coefficients
    S0 = np.asarray(x0_cov, np.float64)
    Qm = np.asarray(Q, np.float64)
    N = np.asarray(A, np.float64) - np.eye(4)
    M1 = N @ S0 + S0 @ N.T
    M2 = N @ S0 @ N.T
    P1 = N @ Qm + Qm @ N.T
    P2 = N @ Qm @ N.T
    C = [S0,
         Qm + M1 - P1 / 2 + P2 / 6,
         M2 + P1 / 2 - P2 / 2,
         P2 / 3]
    # G_j(f) = sum_{k>=j} binom(k,j) C_k f^(k-j), columns (f,ch)-interleaved
    f = np.arange(F, dtype=np.float64)
    binom = np.array([[1, 1, 1, 1], [0, 1, 2, 3], [0, 0, 1, 3], [0, 0, 0, 1]],
                     dtype=np.float64)
    gtab = np.zeros((4, F, 16), np.float64)
    for j in range(4):
        for k in range(j, 4):
            gtab[j] += binom[j, k] * np.power(f, k - j)[:, None] * C[k].reshape(16)[None, :]
    gtab = np.ascontiguousarray(gtab.reshape(4, COVW).astype(np.float32))

    taups = []
    for c in range(NCORES):
        tau = 1.0 + c * L + 977.0 * np.arange(P, dtype=np.float64)
        taups.append(np.ascontiguousarray(
            np.stack([tau ** j for j in range(4)]).astype(np.float32)))
    return vchunks, gtab, taups


def _fused_in_maps(v_sequence, x0_mean, x0_cov, A, Q):
    """Per-core input maps for the fused single-launch kernel."""
    vchunks, gtab, taups = _host_prep(v_sequence, x0_mean, x0_cov, A, Q)
    x0 = np.asarray(x0_mean, np.float64)
    ltri = np.ascontiguousarray(np.triu(np.ones((P, P)), 1).astype(np.float32))
    ones = np.ones((P, 1), np.float32)
    dt = float(DT)
    v0 = x0[2:4]      # initial velocity
    p0 = x0[0:2]      # initial position
    unitr = np.array([[1, 0, 0, 0], [0, 1, 0, 0]], np.float32)
    in_maps = []
    for c in range(NCORES):
        # lhsT blocks [10, 128] each: weights replicated across partitions,
        # rows 8-9 = per-core constants (applied via the rhs unit rows)
        wmb = np.zeros((NCORES + 2, 4 * P), np.float64)
        for cp in range(c):
            wmb[cp, 0:P] = dt                                      # vel bias @ U
            wmb[cp, P:2 * P] = dt * dt                             # ramp coef @ U
            wmb[cp, 2 * P:3 * P] = dt * dt * (L * (c - cp) + 0.5)  # pos bias @ U
            wmb[cp, 3 * P:4 * P] = -dt * dt                        # pos bias @ J
        wmb[NCORES + 0, 0:P] = v0[0]
        wmb[NCORES + 1, 0:P] = v0[1]
        wmb[NCORES + 0, P:2 * P] = dt * v0[0]
        wmb[NCORES + 1, P:2 * P] = dt * v0[1]
        wmb[NCORES + 0, 2 * P:3 * P] = p0[0] + dt * c * L * v0[0]
        wmb[NCORES + 1, 2 * P:3 * P] = p0[1] + dt * c * L * v0[1]
        in_maps.append({
            "v": vchunks[c],
            "gtab": gtab,
            "taup": taups[c],
            "ltri": ltri,
            "ones": ones,
            "wmbig": np.ascontiguousarray(wmb.astype(np.float32)),
            "unitr": unitr,
        })
    return in_maps


def _host_seeds(tots, x0_mean):
    """Combine phase-A per-partition sums into per-partition scan initials.

    tots: list of NCORES arrays [128,4] = [sum u0, sum u1, sum j*u0, sum j*u1]
    Returns per-core [128,4] fp32: [initS_x, initS_y, initR_x, initR_y].
    """
    x0 = np.asarray(x0_mean, np.float64)
    seedS = x0[2:4] / DT            # running scan-1 state (u units)
    seedR = x0[0:2] / (DT * DT)     # running scan-2 state
    inits = []
    for c in range(NCORES):
        tot = np.asarray(tots[c], np.float64)
        U = tot[:, 0:2]             # per-partition sums of u
        J = tot[:, 2:4]             # per-partition sums of j*u (j = 1..F local)
        initS = np.empty((P, 2))
        initR = np.empty((P, 2))
        for p in range(P):
            initS[p] = seedS
            initR[p] = seedR
            flen = float(F if p < P - 1 else L - (P - 1) * F)  # true elements
            # sum over partition of scan-1 states = flen*seedS + sum (flen-j+1) u_j
            # (padded tail elements are zero so they contribute nothing)
            dR = flen * seedS + (flen + 1.0) * U[p] - J[p] - 0.5 * U[p]
            seedS = seedS + U[p]
            seedR = seedR + dR
        inits.append(np.ascontiguousarray(
            np.concatenate([initS, initR], axis=1).astype(np.float32)))
    return inits


def kernel(v_sequence, x0_mean, x0_cov, A, B, Q):
    v_sequence = np.asarray(v_sequence, np.float32)
    x0_mean = np.asarray(x0_mean, np.float32)
    x0_cov = np.asarray(x0_cov, np.float32)
    A = np.asarray(A, np.float32)
    B = np.asarray(B, np.float32)
    Q = np.asarray(Q, np.float32)

    core_ids = list(range(NCORES))
    import os
    if os.environ.get("KFUSED", "1") == "1":
        if "f" not in _CACHE:
            _CACHE["f"] = _build_fused()
        nc_f = _CACHE["f"]
        in_maps = _fused_in_maps(v_sequence, x0_mean, x0_cov, A, Q)
        out_m = run_bass_kernel_spmd(nc_f, in_maps, core_ids, trace=TRACE)
        res_m = out_m.results
        LAST_RESULTS.clear()
        LAST_RESULTS["m"] = out_m
    else:
        nc_a, nc_m = _get_kernels()
        vchunks, gtab, taups = _host_prep(v_sequence, x0_mean, x0_cov, A, Q)
        out_a = run_bass_kernel_spmd(
            nc_a, [{"v": vchunks[c]} for c in core_ids], core_ids, trace=TRACE)
        res_a = out_a.results
        inits = _host_seeds([r["tot"] for r in res_a], x0_mean)
        out_m = run_bass_kernel_spmd(
            nc_m,
            [{"v": vchunks[c], "gtab": gtab, "taup": taups[c], "init": inits[c]}
             for c in core_ids],
            core_ids, trace=TRACE)
        res_m = out_m.results
        LAST_RESULTS.clear()
        LAST_RESULTS["a"] = out_a
        LAST_RESULTS["m"] = out_m

    mean = np.empty((1, T + 1, 4), np.float32)
    cov = np.empty((1, T + 1, 4, 4), np.float32)
    mean[0, 0] = x0_mean
    cov[0, 0] = x0_cov
    for c in range(NCORES):
        mu = res_m[c]["mu_out"].reshape(PADCHUNK, 4)[:L]
        sg = res_m[c]["cov_out"].reshape(PADCHUNK, 16)[:L]
        mean[0, 1 + c * L:1 + (c + 1) * L] = mu
        cov[0, 1 + c * L:1 + (c + 1) * L] = sg.reshape(L, 4, 4)
    return mean, cov


# revision 50
# speedup vs baseline: 1.1233x; 1.0076x over previous
"""Trainium2 Bass kernel for the DoubleIntegrator affine-recurrence scan.

Math reformulation (exact, validated against the sequential reference):

  mu:  vel_t = vel_0 + dt * S_t            with S_t = sum_{k<t} tanh(v_k)
       pos_t = pos_0 + t*dt*vel_0 + dt^2 * R_t
       R_t   = inclusive_scan(S_t - 0.5 * u_{t-1})
       -> two chained prefix scans per control channel.

  cov: A = I + N with N nilpotent (N^2 = 0), so A^k = I + k*N exactly and
       Sigma_t = C0 + C1 t + C2 t^2 + C3 t^3 (4x4 coefficient matrices from
       x0_cov, Q, N).  Expanding t = tau_p + f gives
       Sigma_t = sum_j tau_p^j * G_j(f) -- a K=4 matmul per output tile with a
       host-precomputed f-polynomial table whose columns are already in the
       final (f,ch)-interleaved HBM layout.

Sharding: T=1e6 timesteps split across 8 cores (125000 each, zero-padded to
128*977=125056).  Per core, time is partition-major: partition p holds the
slab [p*977, (p+1)*977).

Default path (KFUSED=1): ONE SPMD launch.  Within-core partition carries come
from a strict-lower-triangular ones matmul on per-partition accumulator sums;
cross-core carries go through a 128-byte AllGather of per-chunk sums, and are
applied OFF the scan critical path as output-time corrections (a constant
shift for velocity, constant+ramp for position) using host-precomputed
per-core weight vectors.  The cov path is fully independent: 31 float32r
K=4 matmuls against a host-built f-polynomial table, PSUM->SBUF copies split
across ACT/DVE by slack, chunked DMA out.  Fallback (KFUSED=0): two launches
(phase-A reduction kernel + host float64 seed combine + main kernel).
"""

import sys

import numpy as np

for _p in ("/opt/trn_rl_repo",):
    if _p not in sys.path:
        sys.path.insert(0, _p)

import concourse.bass as bass
import concourse.mybir as mybir
import concourse.tile as tile
from concourse import bacc
from concourse.bass_utils import run_bass_kernel_spmd


def _install_ntff_shim():
    """Provide antenv.axon_hooks (missing in this image) so trace=True works."""
    try:
        import antenv.axon_hooks  # noqa: F401
        return
    except ImportError:
        pass
    import types
    try:
        import trn_agent_boot.trn_boot as _tb
        hook = _tb._ntff_profile_via_ctypes("/opt/axon/libaxon_pjrt.so")
    except Exception:
        hook = None
    mod = types.ModuleType("antenv.axon_hooks")
    mod.get_axon_ntff_profile_hook = lambda: hook
    sys.modules["antenv.axon_hooks"] = mod


_install_ntff_shim()

F32 = mybir.dt.float32
I32 = mybir.dt.int32

T = 1_000_000
DT = 0.2
NCORES = 8
L = T // NCORES          # 125000 true timesteps per core
P = 128
F = 977                  # free-dim per partition
PADCHUNK = P * F         # 125056 padded timesteps per core
COVW = 16 * F            # 15632 cov columns per partition
MUW = 4 * F              # 3908 mean columns per partition
SIG_CHUNK = 512          # cov matmul/psum chunk (one PSUM bank of fp32)


def _build_phase_a() -> bass.Bass:
    """Per-partition reduction kernel: tot[p] = [sum u0, sum u1, sum j*u0, sum j*u1]
    with u = tanh(v) and j = f+1 the 1-based position within the partition."""
    nc = bacc.Bacc("TRN2", target_bir_lowering=False, debug=False,
                   num_devices=NCORES)
    v = nc.dram_tensor("v", [P, 2 * F], F32, kind="ExternalInput")
    tot = nc.dram_tensor("tot", [P, 4], F32, kind="ExternalOutput")

    HALF = F // 2  # pipeline the chain in two column-halves to overlap DMA
    with tile.TileContext(nc) as tc:
        with tc.tile_pool(name="main", bufs=1) as pool:
            vt = pool.tile([P, 2 * F], F32)
            ramp_i = pool.tile([P, F], I32)
            nc.gpsimd.iota(ramp_i[:], pattern=[[1, F]], base=1, channel_multiplier=0)
            ramp = pool.tile([P, F], F32)
            nc.vector.tensor_copy(ramp[:], ramp_i[:])

            # halves of the raw [f,c]-interleaved input
            nc.sync.dma_start(vt[:, :2 * HALF], v[:, :2 * HALF])
            nc.sync.dma_start(vt[:, 2 * HALF:], v[:, 2 * HALF:])

            tott = pool.tile([P, 4, 2], F32)   # [partition, col, half]
            v3 = vt[:].rearrange("p (f c) -> p f c", c=2)
            us = [pool.tile([P, F], F32, tag=f"u{ch}", name=f"u{ch}")
                  for ch in range(2)]
            dumps = [pool.tile([P, F], F32, tag=f"d{ch}", name=f"d{ch}")
                     for ch in range(2)]
            halves = [(0, HALF), (HALF, F)]
            for h, (f0, f1) in enumerate(halves):
                for ch in range(2):
                    nc.scalar.activation(
                        us[ch][:, f0:f1], v3[:, f0:f1, ch],
                        mybir.ActivationFunctionType.Tanh,
                        accum_out=tott[:, ch, h:h + 1],
                    )
                    eng = nc.vector if ch == 0 else nc.gpsimd
                    eng.scalar_tensor_tensor(
                        out=dumps[ch][:, f0:f1], in0=us[ch][:, f0:f1], scalar=1.0,
                        in1=ramp[:, f0:f1],
                        op0=mybir.AluOpType.mult, op1=mybir.AluOpType.mult,
                        accum_out=tott[:, 2 + ch, h:h + 1],
                    )
            # combine the two half-sums: tot[p, c] = tott[p, c, 0] + tott[p, c, 1]
            tsum = pool.tile([P, 4], F32)
            nc.vector.tensor_tensor(
                out=tsum[:], in0=tott[:, :, 0], in1=tott[:, :, 1],
                op=mybir.AluOpType.add)
            nc.sync.dma_start(tot[:], tsum[:])
    nc.compile()
    return nc


def _build_main() -> bass.Bass:
    """Main kernel: scans for the mean, K=4 matmul polynomial eval for cov."""
    nc = bacc.Bacc("TRN2", target_bir_lowering=False, debug=False,
                   num_devices=NCORES)
    F32R = mybir.dt.float32r
    v = nc.dram_tensor("v", [P, 2 * F], F32, kind="ExternalInput")
    gtab = nc.dram_tensor("gtab", [4, COVW], F32R, kind="ExternalInput")
    taup = nc.dram_tensor("taup", [4, P], F32R, kind="ExternalInput")
    init = nc.dram_tensor("init", [P, 4], F32, kind="ExternalInput")
    mu_out = nc.dram_tensor("mu_out", [P, MUW], F32, kind="ExternalOutput")
    cov_out = nc.dram_tensor("cov_out", [P, COVW], F32, kind="ExternalOutput")

    add = mybir.AluOpType.add
    mult = mybir.AluOpType.mult
    bypass = mybir.AluOpType.bypass

    with tile.TileContext(nc) as tc:
        with (
            tc.tile_pool(name="main", bufs=1) as pool,
            tc.tile_pool(name="sigstage", bufs=6) as stage_pool,
            tc.tile_pool(name="psum", bufs=6, space="PSUM") as psum_pool,
        ):
            # small gating DMAs first so the sigma pipeline starts immediately
            tp = pool.tile([4, P], F32R)
            nc.sync.dma_start(tp[:], taup[:])
            it = pool.tile([P, 4], F32)
            nc.sync.dma_start(it[:], init[:])
            gt = pool.tile([4, COVW], F32R)
            nc.sync.dma_start(gt[:], gtab[:])
            vt = pool.tile([P, 2 * F], F32)
            nc.sync.dma_start(vt[:], v[:])

            # ---- cov: one K=4 matmul per 512-column chunk, PSUM -> SBUF -> HBM
            n_chunks = (COVW + SIG_CHUNK - 1) // SIG_CHUNK
            for k in range(n_chunks):
                c0 = k * SIG_CHUNK
                w = min(SIG_CHUNK, COVW - c0)
                ps = psum_pool.tile([P, SIG_CHUNK], F32, tag="sig")
                # float32r: same fp32 bits, full-rate PE mode (4x fp32) for N>=256
                nc.tensor.matmul(ps[:, :w], tp[:], gt[:, c0:c0 + w],
                                 start=True, stop=True)
                st = stage_pool.tile([P, SIG_CHUNK], F32, tag="st")
                if k < 6:
                    # DVE is idle until v arrives; ACT is busy with the
                    # activation table load + tanh early on
                    nc.vector.tensor_copy(st[:, :w], ps[:, :w])
                else:
                    nc.scalar.copy(st[:, :w], ps[:, :w])
                nc.sync.dma_start(cov_out[:, c0:c0 + w], st[:, :w])

            # ---- mean: tanh, double scan, scaled strided writes
            v3 = vt[:].rearrange("p (f c) -> p f c", c=2)
            muv = pool.tile([P, MUW], F32)
            mu3 = muv[:].rearrange("p (f c) -> p f c", c=4)
            for ch in range(2):
                u = pool.tile([P, F], F32, tag=f"u{ch}")
                nc.scalar.activation(u[:], v3[:, :, ch],
                                     mybir.ActivationFunctionType.Tanh)
                s = pool.tile([P, F], F32, tag=f"s{ch}")
                nc.vector.tensor_tensor_scan(
                    out=s[:], data0=u[:], data1=u[:],
                    initial=it[:, ch:ch + 1], op0=add, op1=bypass)
                w_ = pool.tile([P, F], F32, tag=f"w{ch}")
                nc.vector.scalar_tensor_tensor(
                    out=w_[:], in0=u[:], scalar=-0.5, in1=s[:],
                    op0=mult, op1=add)
                r = pool.tile([P, F], F32, tag=f"r{ch}")
                nc.vector.tensor_tensor_scan(
                    out=r[:], data0=w_[:], data1=w_[:],
                    initial=it[:, 2 + ch:3 + ch], op0=add, op1=bypass)
                # pos = dt^2 * R, vel = dt * S  (seeds already folded in)
                nc.vector.tensor_scalar(
                    out=mu3[:, :, ch], in0=r[:], scalar1=DT * DT, scalar2=None,
                    op0=mult)
                nc.vector.tensor_scalar(
                    out=mu3[:, :, 2 + ch], in0=s[:], scalar1=DT, scalar2=None,
                    op0=mult)
            nc.sync.dma_start(mu_out[:], muv[:])
    nc.compile()
    return nc


def _build_fused() -> bass.Bass:
    """Single-launch kernel.

    The cross-core scan seeds arrive via a tiny AllGather, but they are kept
    OFF the scan critical path: scans run with local (within-core) partition
    carries seeded from an Ltri matmul, and the cross-core seed is applied at
    output time as a constant shift (velocity), and a constant + ramp shift
    (position):

        vel  = dt*S_loc  + [dt*seedS]
        pos  = dt^2*R_loc + [dt^2*seedR] + j * [dt^2*seedS]

    with j the 1-based global-in-chunk index ramp.  All bracketed per-core
    scalars are linear in the AllGathered per-chunk sums with host-provided
    weights (wmask) and constants (consts)."""
    nc = bacc.Bacc("TRN2", target_bir_lowering=False, debug=False,
                   num_devices=NCORES)
    F32R = mybir.dt.float32r
    v = nc.dram_tensor("v", [P, 2 * F], F32, kind="ExternalInput")
    gtab = nc.dram_tensor("gtab", [4, COVW], F32R, kind="ExternalInput")
    taup = nc.dram_tensor("taup", [4, P], F32R, kind="ExternalInput")
    ltri = nc.dram_tensor("ltri", [P, P], F32, kind="ExternalInput")
    ones = nc.dram_tensor("ones", [P, 1], F32, kind="ExternalInput")
    wmbig = nc.dram_tensor("wmbig", [NCORES + 2, 4 * P], F32, kind="ExternalInput")
    unitr = nc.dram_tensor("unitr", [2, 4], F32, kind="ExternalInput")
    mu_out = nc.dram_tensor("mu_out", [P, MUW], F32, kind="ExternalOutput")
    cov_out = nc.dram_tensor("cov_out", [P, COVW], F32, kind="ExternalOutput")

    add = mybir.AluOpType.add
    mult = mybir.AluOpType.mult
    bypass = mybir.AluOpType.bypass
    Copy = mybir.ActivationFunctionType.Copy

    with tile.TileContext(nc) as tc:
        with (
            tc.tile_pool(name="main", bufs=1) as pool,
            tc.tile_pool(name="sigstage", bufs=8) as stage_pool,
            tc.tile_pool(name="psum", bufs=5, space="PSUM") as psum_pool,
            tc.tile_pool(name="spsum", bufs=3, space="PSUM") as spsum_pool,
            tc.tile_pool(name="dram", bufs=2, space="DRAM") as dram_pool,
        ):
            # --- input DMAs: sigma lhsT first, then v (mean chain), the rest
            tp = pool.tile([4, P], F32R)
            nc.sync.dma_start(tp[:], taup[:])
            vt = pool.tile([P, 2 * F], F32)
            HF = F // 2
            nc.sync.dma_start(vt[:, :2 * HF], v[:, :2 * HF])
            nc.sync.dma_start(vt[:, 2 * HF:], v[:, 2 * HF:])
            gt = pool.tile([4, COVW], F32R)
            nc.sync.dma_start(gt[:], gtab[:])
            lt = pool.tile([P, P], F32)
            nc.sync.dma_start(lt[:], ltri[:])
            on = pool.tile([P, 1], F32)
            nc.sync.dma_start(on[:], ones[:])
            wmb = pool.tile([NCORES + 2, 4 * P], F32)
            nc.sync.dma_start(wmb[:], wmbig[:])

            # --- mean pre-path part 1: engines ACT/Pool get these FIRST so
            # tanh is not queued behind the sigma PSUM copies (in-order queues)
            ramp_i = pool.tile([P, F], I32)
            nc.gpsimd.iota(ramp_i[:], pattern=[[1, F]], base=1,
                           channel_multiplier=F)
            ramp = pool.tile([P, F], F32)
            nc.vector.tensor_copy(ramp[:], ramp_i[:])
            v3 = vt[:].rearrange("p (f c) -> p f c", c=2)
            tot = pool.tile([P, 4], F32)     # per-partition [U0,U1,Jg0,Jg1]
            tth = pool.tile([P, 4, 2], F32)  # per-half accumulators
            us = [pool.tile([P, F], F32, tag=f"u{ch}", name=f"u{ch}")
                  for ch in range(2)]
            fhalves = [(0, HF), (HF, F)]
            for h, (f0, f1) in enumerate(fhalves):
                for ch in range(2):
                    nc.scalar.activation(
                        us[ch][:, f0:f1], v3[:, f0:f1, ch],
                        mybir.ActivationFunctionType.Tanh,
                        accum_out=tth[:, ch, h:h + 1])

            # --- sigma chunk emitter (PE-order interleaved with mean matmuls)
            n_chunks = (COVW + SIG_CHUNK - 1) // SIG_CHUNK
            def sig_chunks(k0, k1):
                for k in range(k0, min(k1, n_chunks)):
                    c0 = k * SIG_CHUNK
                    w = min(SIG_CHUNK, COVW - c0)
                    ps = psum_pool.tile([P, SIG_CHUNK], F32, tag="sig",
                                        name=f"ps{k}")
                    nc.tensor.matmul(ps[:, :w], tp[:], gt[:, c0:c0 + w],
                                     start=True, stop=True)
                    st = stage_pool.tile([P, SIG_CHUNK], F32, tag="st",
                                         name=f"st{k}")
                    # drain PSUM on whichever engine has slack: ACT early
                    # (DVE's queue gates the collective via dumps+csum copy),
                    # both late for drain speed
                    if k >= 13 and k % 2 == 0:
                        nc.vector.tensor_copy(st[:, :w], ps[:, :w])
                    else:
                        nc.scalar.copy(st[:, :w], ps[:, :w])
                    nc.sync.dma_start(cov_out[:, c0:c0 + w], st[:, :w])

            sig_chunks(0, 1)

            # --- mean pre-path part 2: chunk summary -> AllGather
            dump = pool.tile([P, F], F32)
            for h, (f0, f1) in enumerate(fhalves):
                for ch in range(2):
                    nc.vector.scalar_tensor_tensor(
                        out=dump[:, f0:f1], in0=us[ch][:, f0:f1], scalar=1.0,
                        in1=ramp[:, f0:f1], op0=mult, op1=mult,
                        accum_out=tth[:, 2 + ch, h:h + 1])
            nc.vector.tensor_tensor(out=tot[:], in0=tth[:, :, 0],
                                    in1=tth[:, :, 1], op=add)
            csum_ps = spsum_pool.tile([P, 4], F32, tag="small", name="csum_t")[0:1, :]
            nc.tensor.matmul(csum_ps[:], on[:], tot[:], start=True, stop=True)
            csum_sb = pool.tile([1, 4], F32)
            nc.vector.tensor_copy(csum_sb[:], csum_ps[:])
            in_bounce = dram_pool.tile([1, 4], F32)
            out_bounce = dram_pool.tile([NCORES, 4], F32)
            nc.gpsimd.dma_start(in_bounce[:], csum_sb[:])
            nc.gpsimd.collective_compute(
                "AllGather", bypass, replica_groups=[list(range(NCORES))],
                ins=[in_bounce.opt()], outs=[out_bounce.opt()])
            # gathered sums land in rows 0..7; rows 8-9 are host unit rows
            # that let the seed matmuls add per-core constants via lhsT
            g8 = pool.tile([NCORES + 2, 4], F32)
            nc.sync.dma_start(g8[NCORES:, :], unitr[:])
            nc.gpsimd.dma_start(g8[:NCORES, :], out_bounce[:])

            # --- local scans (seeded by within-core partition carries only)
            carry1_ps = spsum_pool.tile([P, 2], F32, tag="small", name="c1_t")
            nc.tensor.matmul(carry1_ps[:], lt[:], tot[:, 0:2],
                             start=True, stop=True)
            muv = pool.tile([P, MUW], F32)
            mu3 = muv[:].rearrange("p (f c) -> p f c", c=4)
            totw = pool.tile([P, 2], F32)
            ss = [pool.tile([P, F], F32, tag=f"s{ch}", name=f"s{ch}")
                  for ch in range(2)]
            ws = [pool.tile([P, F], F32, tag=f"w{ch}", name=f"w{ch}")
                  for ch in range(2)]
            rs = [pool.tile([P, F], F32, tag=f"r{ch}", name=f"r{ch}")
                  for ch in range(2)]
            for ch in range(2):
                nc.vector.tensor_tensor_scan(
                    out=ss[ch][:], data0=us[ch][:], data1=us[ch][:],
                    initial=carry1_ps[:, ch:ch + 1], op0=add, op1=bypass)
                nc.vector.scalar_tensor_tensor(
                    out=ws[ch][:], in0=us[ch][:], scalar=-0.5, in1=ss[ch][:],
                    op0=mult, op1=add, accum_out=totw[:, ch:ch + 1])

            sig_chunks(1, 13)

            carry2_ps = spsum_pool.tile([P, 2], F32, tag="small", name="c2_t")
            nc.tensor.matmul(carry2_ps[:], lt[:], totw[:],
                             start=True, stop=True)
            for ch in range(2):
                nc.vector.tensor_tensor_scan(
                    out=rs[ch][:], data0=ws[ch][:], data1=ws[ch][:],
                    initial=carry2_ps[:, ch:ch + 1], op0=add, op1=bypass)

            sig_chunks(13, n_chunks)

            # --- seeds from the gathered table (collective-gated, PE tail):
            # one [128,6] PSUM in a single PE pass -- lhsT columns hold the
            # weight vectors REPLICATED per partition (the matmul IS the
            # broadcast), and lhsT rows 8-9 against the rhs unit rows add the
            # per-core constants.
            bc_ps = spsum_pool.tile([P, 6], F32, tag="small", name="bc_t")
            nc.tensor.matmul(bc_ps[:, 0:2], wmb[:, 0:P], g8[:, 0:2],
                             start=True, stop=True)
            nc.tensor.matmul(bc_ps[:, 2:4], wmb[:, P:2 * P], g8[:, 0:2],
                             start=True, stop=True)
            nc.tensor.matmul(bc_ps[:, 4:6], wmb[:, 2 * P:3 * P], g8[:, 0:2],
                             start=True, stop=False)
            nc.tensor.matmul(bc_ps[:, 4:6], wmb[:, 3 * P:4 * P], g8[:, 2:4],
                             start=False, stop=True)
            bc = pool.tile([P, 6], F32)
            nc.vector.tensor_copy(bc[:], bc_ps[:])

            # --- outputs with cross-core corrections; quartered for DMA overlap
            tmps = [pool.tile([P, F], F32, tag=f"t{ch}", name=f"t{ch}")
                    for ch in range(2)]
            NQ = 8
            QW = (F + NQ - 1) // NQ
            for q in range(NQ):
                f0, f1 = q * QW, min((q + 1) * QW, F)
                for ch in range(2):
                    # vel = dt*S_loc + bias(sA)  (GPSIMD: parallel to DVE pos chain)
                    nc.gpsimd.tensor_scalar(
                        mu3[:, f0:f1, 2 + ch], ss[ch][:, f0:f1],
                        DT, bc[:, ch:ch + 1], op0=mult, op1=add)
                    # pos = dt^2*R_loc + bias(sR) + ramp*coeff(sB)
                    nc.vector.tensor_scalar(
                        tmps[ch][:, f0:f1], rs[ch][:, f0:f1],
                        DT * DT, bc[:, 4 + ch:5 + ch], op0=mult, op1=add)
                    nc.vector.scalar_tensor_tensor(
                        out=mu3[:, f0:f1, ch], in0=ramp[:, f0:f1],
                        scalar=bc[:, 2 + ch:3 + ch], in1=tmps[ch][:, f0:f1],
                        op0=mult, op1=add)
                nc.sync.dma_start(mu_out[:, 4 * f0:4 * f1], muv[:, 4 * f0:4 * f1])
    nc.compile()
    return nc


_CACHE: dict = {}
TRACE = False          # set True by test harness to collect NTFF profiles
LAST_RESULTS = {}      # phase results stashed here for the harness


def _get_kernels():
    if "a" not in _CACHE:
        _CACHE["a"] = _build_phase_a()
        _CACHE["m"] = _build_main()
    return _CACHE["a"], _CACHE["m"]


def _host_prep(v_sequence, x0_mean, x0_cov, A, Q):
    """Everything the host precomputes: padded/reshaped per-core v chunks and
    the cov polynomial tables (float64 -> fp32)."""
    v = np.ascontiguousarray(np.asarray(v_sequence, np.float32))
    vpad = np.zeros((NCORES, PADCHUNK, 2), np.float32)
    vpad[:, :L] = v.reshape(NCORES, L, 2)
    vchunks = [np.ascontiguousarray(vpad[c].reshape(P, 2 * F)) for c in range(NCORES)]

    # cov cubic coefficients
    S0 = np.asarray(x0_cov, np.float64)
    Qm = np.asarray(Q, np.float64)
    N = np.asarray(A, np.float64) - np.eye(4)
    M1 = N @ S0 + S0 @ N.T
    M2 = N @ S0 @ N.T
    P1 = N @ Qm + Qm @ N.T
    P2 = N @ Qm @ N.T
    C = [S0,
         Qm + M1 - P1 / 2 + P2 / 6,
         M2 + P1 / 2 - P2 / 2,
         P2 / 3]
    # G_j(f) = sum_{k>=j} binom(k,j) C_k f^(k-j), columns (f,ch)-interleaved
    f = np.arange(F, dtype=np.float64)
    binom = np.array([[1, 1, 1, 1], [0, 1, 2, 3], [0, 0, 1, 3], [0, 0, 0, 1]],
                     dtype=np.float64)
    gtab = np.zeros((4, F, 16), np.float64)
    for j in range(4):
        for k in range(j, 4):
            gtab[j] += binom[j, k] * np.power(f, k - j)[:, None] * C[k].reshape(16)[None, :]
    gtab = np.ascontiguousarray(gtab.reshape(4, COVW).astype(np.float32))

    taups = []
    for c in range(NCORES):
        tau = 1.0 + c * L + 977.0 * np.arange(P, dtype=np.float64)
        taups.append(np.ascontiguousarray(
            np.stack([tau ** j for j in range(4)]).astype(np.float32)))
    return vchunks, gtab, taups


def _fused_in_maps(v_sequence, x0_mean, x0_cov, A, Q):
    """Per-core input maps for the fused single-launch kernel."""
    vchunks, gtab, taups = _host_prep(v_sequence, x0_mean, x0_cov, A, Q)
    x0 = np.asarray(x0_mean, np.float64)
    ltri = np.ascontiguousarray(np.triu(np.ones((P, P)), 1).astype(np.float32))
    ones = np.ones((P, 1), np.float32)
    dt = float(DT)
    v0 = x0[2:4]      # initial velocity
    p0 = x0[0:2]      # initial position
    unitr = np.array([[1, 0, 0, 0], [0, 1, 0, 0]], np.float32)
    in_maps = []
    for c in range(NCORES):
        # lhsT blocks [10, 128] each: weights replicated across partitions,
        # rows 8-9 = per-core constants (applied via the rhs unit rows)
        wmb = np.zeros((NCORES + 2, 4 * P), np.float64)
        for cp in range(c):
            wmb[cp, 0:P] = dt                                      # vel bias @ U
            wmb[cp, P:2 * P] = dt * dt                             # ramp coef @ U
            wmb[cp, 2 * P:3 * P] = dt * dt * (L * (c - cp) + 0.5)  # pos bias @ U
            wmb[cp, 3 * P:4 * P] = -dt * dt                        # pos bias @ J
        wmb[NCORES + 0, 0:P] = v0[0]
        wmb[NCORES + 1, 0:P] = v0[1]
        wmb[NCORES + 0, P:2 * P] = dt * v0[0]
        wmb[NCORES + 1, P:2 * P] = dt * v0[1]
        wmb[NCORES + 0, 2 * P:3 * P] = p0[0] + dt * c * L * v0[0]
        wmb[NCORES + 1, 2 * P:3 * P] = p0[1] + dt * c * L * v0[1]
        in_maps.append({
            "v": vchunks[c],
            "gtab": gtab,
            "taup": taups[c],
            "ltri": ltri,
            "ones": ones,
            "wmbig": np.ascontiguousarray(wmb.astype(np.float32)),
            "unitr": unitr,
        })
    return in_maps


def _host_seeds(tots, x0_mean):
    """Combine phase-A per-partition sums into per-partition scan initials.

    tots: list of NCORES arrays [128,4] = [sum u0, sum u1, sum j*u0, sum j*u1]
    Returns per-core [128,4] fp32: [initS_x, initS_y, initR_x, initR_y].
    """
    x0 = np.asarray(x0_mean, np.float64)
    seedS = x0[2:4] / DT            # running scan-1 state (u units)
    seedR = x0[0:2] / (DT * DT)     # running scan-2 state
    inits = []
    for c in range(NCORES):
        tot = np.asarray(tots[c], np.float64)
        U = tot[:, 0:2]             # per-partition sums of u
        J = tot[:, 2:4]             # per-partition sums of j*u (j = 1..F local)
        initS = np.empty((P, 2))
        initR = np.empty((P, 2))
        for p in range(P):
            initS[p] = seedS
            initR[p] = seedR
            flen = float(F if p < P - 1 else L - (P - 1) * F)  # true elements
            # sum over partition of scan-1 states = flen*seedS + sum (flen-j+1) u_j
            # (padded tail elements are zero so they contribute nothing)
            dR = flen * seedS + (flen + 1.0) * U[p] - J[p] - 0.5 * U[p]
            seedS = seedS + U[p]
            seedR = seedR + dR
        inits.append(np.ascontiguousarray(
            np.concatenate([initS, initR], axis=1).astype(np.float32)))
    return inits


def kernel(v_sequence, x0_mean, x0_cov, A, B, Q):
    v_sequence = np.asarray(v_sequence, np.float32)
    x0_mean = np.asarray(x0_mean, np.float32)
    x0_cov = np.asarray(x0_cov, np.float32)
    A = np.asarray(A, np.float32)
    B = np.asarray(B, np.float32)
    Q = np.asarray(Q, np.float32)

    core_ids = list(range(NCORES))
    import os
    if os.environ.get("KFUSED", "1") == "1":
        if "f" not in _CACHE:
            _CACHE["f"] = _build_fused()
        nc_f = _CACHE["f"]
        in_maps = _fused_in_maps(v_sequence, x0_mean, x0_cov, A, Q)
        out_m = run_bass_kernel_spmd(nc_f, in_maps, core_ids, trace=TRACE)
        res_m = out_m.results
        LAST_RESULTS.clear()
        LAST_RESULTS["m"] = out_m
    else:
        nc_a, nc_m = _get_kernels()
        vchunks, gtab, taups = _host_prep(v_sequence, x0_mean, x0_cov, A, Q)
        out_a = run_bass_kernel_spmd(
            nc_a, [{"v": vchunks[c]} for c in core_ids], core_ids, trace=TRACE)
        res_a = out_a.results
        inits = _host_seeds([r["tot"] for r in res_a], x0_mean)
        out_m = run_bass_kernel_spmd(
            nc_m,
            [{"v": vchunks[c], "gtab": gtab, "taup": taups[c], "init": inits[c]}
             for c in core_ids],
            core_ids, trace=TRACE)
        res_m = out_m.results
        LAST_RESULTS.clear()
        LAST_RESULTS["a"] = out_a
        LAST_RESULTS["m"] = out_m

    mean = np.empty((1, T + 1, 4), np.float32)
    cov = np.empty((1, T + 1, 4, 4), np.float32)
    mean[0, 0] = x0_mean
    cov[0, 0] = x0_cov
    for c in range(NCORES):
        mu = res_m[c]["mu_out"].reshape(PADCHUNK, 4)[:L]
        sg = res_m[c]["cov_out"].reshape(PADCHUNK, 16)[:L]
        mean[0, 1 + c * L:1 + (c + 1) * L] = mu
        cov[0, 1 + c * L:1 + (c + 1) * L] = sg.reshape(L, 4, 4)
    return mean, cov


# revision 51
# speedup vs baseline: 1.1505x; 1.0242x over previous
"""Trainium2 Bass kernel for the DoubleIntegrator affine-recurrence scan.

Math reformulation (exact, validated against the sequential reference):

  mu:  vel_t = vel_0 + dt * S_t            with S_t = sum_{k<t} tanh(v_k)
       pos_t = pos_0 + t*dt*vel_0 + dt^2 * R_t
       R_t   = inclusive_scan(S_t - 0.5 * u_{t-1})
       -> two chained prefix scans per control channel.

  cov: A = I + N with N nilpotent (N^2 = 0), so A^k = I + k*N exactly and
       Sigma_t = C0 + C1 t + C2 t^2 + C3 t^3 (4x4 coefficient matrices from
       x0_cov, Q, N).  Expanding t = tau_p + f gives
       Sigma_t = sum_j tau_p^j * G_j(f) -- a K=4 matmul per output tile with a
       host-precomputed f-polynomial table whose columns are already in the
       final (f,ch)-interleaved HBM layout.

Sharding: T=1e6 timesteps split across 8 cores (125000 each, zero-padded to
128*977=125056).  Per core, time is partition-major: partition p holds the
slab [p*977, (p+1)*977).

Default path (KFUSED=1): ONE SPMD launch.  Within-core partition carries come
from a strict-lower-triangular ones matmul on per-partition accumulator sums;
cross-core carries go through a 128-byte AllGather of per-chunk sums, and are
applied OFF the scan critical path as output-time corrections (a constant
shift for velocity, constant+ramp for position) using host-precomputed
per-core weight vectors.  The cov path is fully independent: 31 float32r
K=4 matmuls against a host-built f-polynomial table, PSUM->SBUF copies split
across ACT/DVE by slack, chunked DMA out.  Fallback (KFUSED=0): two launches
(phase-A reduction kernel + host float64 seed combine + main kernel).
"""

import sys

import numpy as np

for _p in ("/opt/trn_rl_repo",):
    if _p not in sys.path:
        sys.path.insert(0, _p)

import concourse.bass as bass
import concourse.mybir as mybir
import concourse.tile as tile
from concourse import bacc
from concourse.bass_utils import run_bass_kernel_spmd


def _install_ntff_shim():
    """Provide antenv.axon_hooks (missing in this image) so trace=True works."""
    try:
        import antenv.axon_hooks  # noqa: F401
        return
    except ImportError:
        pass
    import types
    try:
        import trn_agent_boot.trn_boot as _tb
        hook = _tb._ntff_profile_via_ctypes("/opt/axon/libaxon_pjrt.so")
    except Exception:
        hook = None
    mod = types.ModuleType("antenv.axon_hooks")
    mod.get_axon_ntff_profile_hook = lambda: hook
    sys.modules["antenv.axon_hooks"] = mod


_install_ntff_shim()

F32 = mybir.dt.float32
I32 = mybir.dt.int32

T = 1_000_000
DT = 0.2
NCORES = 8
L = T // NCORES          # 125000 true timesteps per core
P = 128
F = 977                  # free-dim per partition
PADCHUNK = P * F         # 125056 padded timesteps per core
COVW = 16 * F            # 15632 cov columns per partition
MUW = 4 * F              # 3908 mean columns per partition
SIG_CHUNK = 512          # cov matmul/psum chunk (one PSUM bank of fp32)


def _build_phase_a() -> bass.Bass:
    """Per-partition reduction kernel: tot[p] = [sum u0, sum u1, sum j*u0, sum j*u1]
    with u = tanh(v) and j = f+1 the 1-based position within the partition."""
    nc = bacc.Bacc("TRN2", target_bir_lowering=False, debug=False,
                   num_devices=NCORES)
    v = nc.dram_tensor("v", [P, 2 * F], F32, kind="ExternalInput")
    tot = nc.dram_tensor("tot", [P, 4], F32, kind="ExternalOutput")

    HALF = F // 2  # pipeline the chain in two column-halves to overlap DMA
    with tile.TileContext(nc) as tc:
        with tc.tile_pool(name="main", bufs=1) as pool:
            vt = pool.tile([P, 2 * F], F32)
            ramp_i = pool.tile([P, F], I32)
            nc.gpsimd.iota(ramp_i[:], pattern=[[1, F]], base=1, channel_multiplier=0)
            ramp = pool.tile([P, F], F32)
            nc.vector.tensor_copy(ramp[:], ramp_i[:])

            # halves of the raw [f,c]-interleaved input
            nc.sync.dma_start(vt[:, :2 * HALF], v[:, :2 * HALF])
            nc.sync.dma_start(vt[:, 2 * HALF:], v[:, 2 * HALF:])

            tott = pool.tile([P, 4, 2], F32)   # [partition, col, half]
            v3 = vt[:].rearrange("p (f c) -> p f c", c=2)
            us = [pool.tile([P, F], F32, tag=f"u{ch}", name=f"u{ch}")
                  for ch in range(2)]
            dumps = [pool.tile([P, F], F32, tag=f"d{ch}", name=f"d{ch}")
                     for ch in range(2)]
            halves = [(0, HALF), (HALF, F)]
            for h, (f0, f1) in enumerate(halves):
                for ch in range(2):
                    nc.scalar.activation(
                        us[ch][:, f0:f1], v3[:, f0:f1, ch],
                        mybir.ActivationFunctionType.Tanh,
                        accum_out=tott[:, ch, h:h + 1],
                    )
                    eng = nc.vector if ch == 0 else nc.gpsimd
                    eng.scalar_tensor_tensor(
                        out=dumps[ch][:, f0:f1], in0=us[ch][:, f0:f1], scalar=1.0,
                        in1=ramp[:, f0:f1],
                        op0=mybir.AluOpType.mult, op1=mybir.AluOpType.mult,
                        accum_out=tott[:, 2 + ch, h:h + 1],
                    )
            # combine the two half-sums: tot[p, c] = tott[p, c, 0] + tott[p, c, 1]
            tsum = pool.tile([P, 4], F32)
            nc.vector.tensor_tensor(
                out=tsum[:], in0=tott[:, :, 0], in1=tott[:, :, 1],
                op=mybir.AluOpType.add)
            nc.sync.dma_start(tot[:], tsum[:])
    nc.compile()
    return nc


def _build_main() -> bass.Bass:
    """Main kernel: scans for the mean, K=4 matmul polynomial eval for cov."""
    nc = bacc.Bacc("TRN2", target_bir_lowering=False, debug=False,
                   num_devices=NCORES)
    F32R = mybir.dt.float32r
    v = nc.dram_tensor("v", [P, 2 * F], F32, kind="ExternalInput")
    gtab = nc.dram_tensor("gtab", [4, COVW], F32R, kind="ExternalInput")
    taup = nc.dram_tensor("taup", [4, P], F32R, kind="ExternalInput")
    init = nc.dram_tensor("init", [P, 4], F32, kind="ExternalInput")
    mu_out = nc.dram_tensor("mu_out", [P, MUW], F32, kind="ExternalOutput")
    cov_out = nc.dram_tensor("cov_out", [P, COVW], F32, kind="ExternalOutput")

    add = mybir.AluOpType.add
    mult = mybir.AluOpType.mult
    bypass = mybir.AluOpType.bypass

    with tile.TileContext(nc) as tc:
        with (
            tc.tile_pool(name="main", bufs=1) as pool,
            tc.tile_pool(name="sigstage", bufs=6) as stage_pool,
            tc.tile_pool(name="psum", bufs=6, space="PSUM") as psum_pool,
        ):
            # small gating DMAs first so the sigma pipeline starts immediately
            tp = pool.tile([4, P], F32R)
            nc.sync.dma_start(tp[:], taup[:])
            it = pool.tile([P, 4], F32)
            nc.sync.dma_start(it[:], init[:])
            gt = pool.tile([4, COVW], F32R)
            nc.sync.dma_start(gt[:], gtab[:])
            vt = pool.tile([P, 2 * F], F32)
            nc.sync.dma_start(vt[:], v[:])

            # ---- cov: one K=4 matmul per 512-column chunk, PSUM -> SBUF -> HBM
            n_chunks = (COVW + SIG_CHUNK - 1) // SIG_CHUNK
            for k in range(n_chunks):
                c0 = k * SIG_CHUNK
                w = min(SIG_CHUNK, COVW - c0)
                ps = psum_pool.tile([P, SIG_CHUNK], F32, tag="sig")
                # float32r: same fp32 bits, full-rate PE mode (4x fp32) for N>=256
                nc.tensor.matmul(ps[:, :w], tp[:], gt[:, c0:c0 + w],
                                 start=True, stop=True)
                st = stage_pool.tile([P, SIG_CHUNK], F32, tag="st")
                if k < 6:
                    # DVE is idle until v arrives; ACT is busy with the
                    # activation table load + tanh early on
                    nc.vector.tensor_copy(st[:, :w], ps[:, :w])
                else:
                    nc.scalar.copy(st[:, :w], ps[:, :w])
                nc.sync.dma_start(cov_out[:, c0:c0 + w], st[:, :w])

            # ---- mean: tanh, double scan, scaled strided writes
            v3 = vt[:].rearrange("p (f c) -> p f c", c=2)
            muv = pool.tile([P, MUW], F32)
            mu3 = muv[:].rearrange("p (f c) -> p f c", c=4)
            for ch in range(2):
                u = pool.tile([P, F], F32, tag=f"u{ch}")
                nc.scalar.activation(u[:], v3[:, :, ch],
                                     mybir.ActivationFunctionType.Tanh)
                s = pool.tile([P, F], F32, tag=f"s{ch}")
                nc.vector.tensor_tensor_scan(
                    out=s[:], data0=u[:], data1=u[:],
                    initial=it[:, ch:ch + 1], op0=add, op1=bypass)
                w_ = pool.tile([P, F], F32, tag=f"w{ch}")
                nc.vector.scalar_tensor_tensor(
                    out=w_[:], in0=u[:], scalar=-0.5, in1=s[:],
                    op0=mult, op1=add)
                r = pool.tile([P, F], F32, tag=f"r{ch}")
                nc.vector.tensor_tensor_scan(
                    out=r[:], data0=w_[:], data1=w_[:],
                    initial=it[:, 2 + ch:3 + ch], op0=add, op1=bypass)
                # pos = dt^2 * R, vel = dt * S  (seeds already folded in)
                nc.vector.tensor_scalar(
                    out=mu3[:, :, ch], in0=r[:], scalar1=DT * DT, scalar2=None,
                    op0=mult)
                nc.vector.tensor_scalar(
                    out=mu3[:, :, 2 + ch], in0=s[:], scalar1=DT, scalar2=None,
                    op0=mult)
            nc.sync.dma_start(mu_out[:], muv[:])
    nc.compile()
    return nc


def _build_fused() -> bass.Bass:
    """Single-launch kernel.

    The cross-core scan seeds arrive via a tiny AllGather, but they are kept
    OFF the scan critical path: scans run with local (within-core) partition
    carries seeded from an Ltri matmul, and the cross-core seed is applied at
    output time as a constant shift (velocity), and a constant + ramp shift
    (position):

        vel  = dt*S_loc  + [dt*seedS]
        pos  = dt^2*R_loc + [dt^2*seedR] + j * [dt^2*seedS]

    with j the 1-based global-in-chunk index ramp.  All bracketed per-core
    scalars are linear in the AllGathered per-chunk sums with host-provided
    weights (wmask) and constants (consts)."""
    nc = bacc.Bacc("TRN2", target_bir_lowering=False, debug=False,
                   num_devices=NCORES)
    F32R = mybir.dt.float32r
    v = nc.dram_tensor("v", [P, 2 * F], F32, kind="ExternalInput")
    gtab = nc.dram_tensor("gtab", [4, COVW], F32R, kind="ExternalInput")
    taup = nc.dram_tensor("taup", [4, P], F32R, kind="ExternalInput")
    ltri = nc.dram_tensor("ltri", [P, P], F32, kind="ExternalInput")
    ones = nc.dram_tensor("ones", [P, 1], F32, kind="ExternalInput")
    wmbig = nc.dram_tensor("wmbig", [NCORES + 2, 4 * P], F32, kind="ExternalInput")
    unitr = nc.dram_tensor("unitr", [2, 4], F32, kind="ExternalInput")
    mu_out = nc.dram_tensor("mu_out", [P, MUW], F32, kind="ExternalOutput")
    cov_out = nc.dram_tensor("cov_out", [P, COVW], F32, kind="ExternalOutput")

    add = mybir.AluOpType.add
    mult = mybir.AluOpType.mult
    bypass = mybir.AluOpType.bypass
    Copy = mybir.ActivationFunctionType.Copy

    with tile.TileContext(nc) as tc:
        with (
            tc.tile_pool(name="main", bufs=1) as pool,
            tc.tile_pool(name="sigstage", bufs=8) as stage_pool,
            tc.tile_pool(name="psum", bufs=5, space="PSUM") as psum_pool,
            tc.tile_pool(name="spsum", bufs=3, space="PSUM") as spsum_pool,
            tc.tile_pool(name="dram", bufs=2, space="DRAM") as dram_pool,
        ):
            # --- input DMAs: sigma lhsT first, then v (mean chain), the rest
            tp = pool.tile([4, P], F32R)
            nc.sync.dma_start(tp[:], taup[:])
            vt = pool.tile([P, 2 * F], F32)
            HF = F // 2
            nc.sync.dma_start(vt[:, :2 * HF], v[:, :2 * HF])
            nc.sync.dma_start(vt[:, 2 * HF:], v[:, 2 * HF:])
            gt = pool.tile([4, COVW], F32R)
            nc.sync.dma_start(gt[:], gtab[:])
            lt = pool.tile([P, P], F32)
            nc.sync.dma_start(lt[:], ltri[:])
            on = pool.tile([P, 1], F32)
            nc.sync.dma_start(on[:], ones[:])
            wmb = pool.tile([NCORES + 2, 4 * P], F32)
            nc.sync.dma_start(wmb[:], wmbig[:])

            # --- mean pre-path part 1: engines ACT/Pool get these FIRST so
            # tanh is not queued behind the sigma PSUM copies (in-order queues)
            ramp_i = pool.tile([P, F], I32)
            nc.gpsimd.iota(ramp_i[:], pattern=[[1, F]], base=1,
                           channel_multiplier=F)
            ramp = pool.tile([P, F], F32)
            nc.vector.tensor_copy(ramp[:], ramp_i[:])
            v3 = vt[:].rearrange("p (f c) -> p f c", c=2)
            tot = pool.tile([P, 4], F32)     # per-partition [U0,U1,Jg0,Jg1]
            tth = pool.tile([P, 4, 2], F32)  # per-half accumulators
            us = [pool.tile([P, F], F32, tag=f"u{ch}", name=f"u{ch}")
                  for ch in range(2)]
            fhalves = [(0, HF), (HF, F)]
            for h, (f0, f1) in enumerate(fhalves):
                for ch in range(2):
                    nc.scalar.activation(
                        us[ch][:, f0:f1], v3[:, f0:f1, ch],
                        mybir.ActivationFunctionType.Tanh,
                        accum_out=tth[:, ch, h:h + 1])

            # --- sigma chunk emitter (PE-order interleaved with mean matmuls)
            n_chunks = (COVW + SIG_CHUNK - 1) // SIG_CHUNK
            def sig_chunks(k0, k1):
                for k in range(k0, min(k1, n_chunks)):
                    c0 = k * SIG_CHUNK
                    w = min(SIG_CHUNK, COVW - c0)
                    ps = psum_pool.tile([P, SIG_CHUNK], F32, tag="sig",
                                        name=f"ps{k}")
                    nc.tensor.matmul(ps[:, :w], tp[:], gt[:, c0:c0 + w],
                                     start=True, stop=True)
                    st = stage_pool.tile([P, SIG_CHUNK], F32, tag="st",
                                         name=f"st{k}")
                    # drain PSUM on whichever engine has slack: ACT early
                    # (DVE's queue gates the collective via dumps+csum copy),
                    # both late for drain speed
                    if k >= 13 and k % 2 == 0:
                        nc.vector.tensor_copy(st[:, :w], ps[:, :w])
                    else:
                        nc.scalar.copy(st[:, :w], ps[:, :w])
                    nc.sync.dma_start(cov_out[:, c0:c0 + w], st[:, :w])

            sig_chunks(0, 1)

            # --- mean pre-path part 2: chunk summary -> AllGather
            dump = pool.tile([P, F], F32)
            for h, (f0, f1) in enumerate(fhalves):
                for ch in range(2):
                    nc.vector.scalar_tensor_tensor(
                        out=dump[:, f0:f1], in0=us[ch][:, f0:f1], scalar=1.0,
                        in1=ramp[:, f0:f1], op0=mult, op1=mult,
                        accum_out=tth[:, 2 + ch, h:h + 1])
            nc.vector.tensor_tensor(out=tot[:], in0=tth[:, :, 0],
                                    in1=tth[:, :, 1], op=add)
            csum_ps = spsum_pool.tile([P, 4], F32, tag="small", name="csum_t")[0:1, :]
            nc.tensor.matmul(csum_ps[:], on[:], tot[:], start=True, stop=True)
            csum_sb = pool.tile([1, 4], F32)
            nc.vector.tensor_copy(csum_sb[:], csum_ps[:])
            in_bounce = dram_pool.tile([1, 4], F32)
            out_bounce = dram_pool.tile([NCORES, 4], F32)
            # HWDGE (~1us) instead of SWDGE (~2us): only slack-rich cov DMAs
            # queue behind this wait in the SP stream
            nc.sync.dma_start(in_bounce[:], csum_sb[:])
            nc.gpsimd.collective_compute(
                "AllGather", bypass, replica_groups=[list(range(NCORES))],
                ins=[in_bounce.opt()], outs=[out_bounce.opt()])
            # gathered sums land in rows 0..7; rows 8-9 are host unit rows
            # that let the seed matmuls add per-core constants via lhsT
            g8 = pool.tile([NCORES + 2, 4], F32)
            nc.sync.dma_start(g8[NCORES:, :], unitr[:])
            nc.gpsimd.dma_start(g8[:NCORES, :], out_bounce[:])

            # --- local scans (seeded by within-core partition carries only)
            carry1_ps = spsum_pool.tile([P, 2], F32, tag="small", name="c1_t")
            nc.tensor.matmul(carry1_ps[:], lt[:], tot[:, 0:2],
                             start=True, stop=True)
            muv = pool.tile([P, MUW], F32)
            mu3 = muv[:].rearrange("p (f c) -> p f c", c=4)
            totw = pool.tile([P, 2], F32)
            ss = [pool.tile([P, F], F32, tag=f"s{ch}", name=f"s{ch}")
                  for ch in range(2)]
            ws = [pool.tile([P, F], F32, tag=f"w{ch}", name=f"w{ch}")
                  for ch in range(2)]
            rs = [pool.tile([P, F], F32, tag=f"r{ch}", name=f"r{ch}")
                  for ch in range(2)]
            for ch in range(2):
                nc.vector.tensor_tensor_scan(
                    out=ss[ch][:], data0=us[ch][:], data1=us[ch][:],
                    initial=carry1_ps[:, ch:ch + 1], op0=add, op1=bypass)
                nc.vector.scalar_tensor_tensor(
                    out=ws[ch][:], in0=us[ch][:], scalar=-0.5, in1=ss[ch][:],
                    op0=mult, op1=add, accum_out=totw[:, ch:ch + 1])

            sig_chunks(1, 13)

            carry2_ps = spsum_pool.tile([P, 2], F32, tag="small", name="c2_t")
            nc.tensor.matmul(carry2_ps[:], lt[:], totw[:],
                             start=True, stop=True)
            for ch in range(2):
                nc.vector.tensor_tensor_scan(
                    out=rs[ch][:], data0=ws[ch][:], data1=ws[ch][:],
                    initial=carry2_ps[:, ch:ch + 1], op0=add, op1=bypass)

            sig_chunks(13, n_chunks)

            # --- seeds from the gathered table (collective-gated, PE tail):
            # one [128,6] PSUM in a single PE pass -- lhsT columns hold the
            # weight vectors REPLICATED per partition (the matmul IS the
            # broadcast), and lhsT rows 8-9 against the rhs unit rows add the
            # per-core constants.
            bc_ps = spsum_pool.tile([P, 6], F32, tag="small", name="bc_t")
            nc.tensor.matmul(bc_ps[:, 0:2], wmb[:, 0:P], g8[:, 0:2],
                             start=True, stop=True)
            nc.tensor.matmul(bc_ps[:, 2:4], wmb[:, P:2 * P], g8[:, 0:2],
                             start=True, stop=True)
            nc.tensor.matmul(bc_ps[:, 4:6], wmb[:, 2 * P:3 * P], g8[:, 0:2],
                             start=True, stop=False)
            nc.tensor.matmul(bc_ps[:, 4:6], wmb[:, 3 * P:4 * P], g8[:, 2:4],
                             start=False, stop=True)
            bc = pool.tile([P, 6], F32)
            nc.vector.tensor_copy(bc[:], bc_ps[:])

            # --- outputs with cross-core corrections; quartered for DMA overlap
            tmps = [pool.tile([P, F], F32, tag=f"t{ch}", name=f"t{ch}")
                    for ch in range(2)]
            NQ = 8
            QW = (F + NQ - 1) // NQ
            for q in range(NQ):
                f0, f1 = q * QW, min((q + 1) * QW, F)
                for ch in range(2):
                    # vel = dt*S_loc + bias(sA)  (GPSIMD: parallel to DVE pos chain)
                    nc.gpsimd.tensor_scalar(
                        mu3[:, f0:f1, 2 + ch], ss[ch][:, f0:f1],
                        DT, bc[:, ch:ch + 1], op0=mult, op1=add)
                    # pos = dt^2*R_loc + bias(sR) + ramp*coeff(sB)
                    nc.vector.tensor_scalar(
                        tmps[ch][:, f0:f1], rs[ch][:, f0:f1],
                        DT * DT, bc[:, 4 + ch:5 + ch], op0=mult, op1=add)
                    nc.vector.scalar_tensor_tensor(
                        out=mu3[:, f0:f1, ch], in0=ramp[:, f0:f1],
                        scalar=bc[:, 2 + ch:3 + ch], in1=tmps[ch][:, f0:f1],
                        op0=mult, op1=add)
                nc.sync.dma_start(mu_out[:, 4 * f0:4 * f1], muv[:, 4 * f0:4 * f1])
    nc.compile()
    return nc


_CACHE: dict = {}
TRACE = False          # set True by test harness to collect NTFF profiles
LAST_RESULTS = {}      # phase results stashed here for the harness


def _get_kernels():
    if "a" not in _CACHE:
        _CACHE["a"] = _build_phase_a()
        _CACHE["m"] = _build_main()
    return _CACHE["a"], _CACHE["m"]


def _host_prep(v_sequence, x0_mean, x0_cov, A, Q):
    """Everything the host precomputes: padded/reshaped per-core v chunks and
    the cov polynomial tables (float64 -> fp32)."""
    v = np.ascontiguousarray(np.asarray(v_sequence, np.float32))
    vpad = np.zeros((NCORES, PADCHUNK, 2), np.float32)
    vpad[:, :L] = v.reshape(NCORES, L, 2)
    vchunks = [np.ascontiguousarray(vpad[c].reshape(P, 2 * F)) for c in range(NCORES)]

    # cov cubic coefficients
    S0 = np.asarray(x0_cov, np.float64)
    Qm = np.asarray(Q, np.float64)
    N = np.asarray(A, np.float64) - np.eye(4)
    M1 = N @ S0 + S0 @ N.T
    M2 = N @ S0 @ N.T
    P1 = N @ Qm + Qm @ N.T
    P2 = N @ Qm @ N.T
    C = [S0,
         Qm + M1 - P1 / 2 + P2 / 6,
         M2 + P1 / 2 - P2 / 2,
         P2 / 3]
    # G_j(f) = sum_{k>=j} binom(k,j) C_k f^(k-j), columns (f,ch)-interleaved
    f = np.arange(F, dtype=np.float64)
    binom = np.array([[1, 1, 1, 1], [0, 1, 2, 3], [0, 0, 1, 3], [0, 0, 0, 1]],
                     dtype=np.float64)
    gtab = np.zeros((4, F, 16), np.float64)
    for j in range(4):
        for k in range(j, 4):
            gtab[j] += binom[j, k] * np.power(f, k - j)[:, None] * C[k].reshape(16)[None, :]
    gtab = np.ascontiguousarray(gtab.reshape(4, COVW).astype(np.float32))

    taups = []
    for c in range(NCORES):
        tau = 1.0 + c * L + 977.0 * np.arange(P, dtype=np.float64)
        taups.append(np.ascontiguousarray(
            np.stack([tau ** j for j in range(4)]).astype(np.float32)))
    return vchunks, gtab, taups


def _fused_in_maps(v_sequence, x0_mean, x0_cov, A, Q):
    """Per-core input maps for the fused single-launch kernel."""
    vchunks, gtab, taups = _host_prep(v_sequence, x0_mean, x0_cov, A, Q)
    x0 = np.asarray(x0_mean, np.float64)
    ltri = np.ascontiguousarray(np.triu(np.ones((P, P)), 1).astype(np.float32))
    ones = np.ones((P, 1), np.float32)
    dt = float(DT)
    v0 = x0[2:4]      # initial velocity
    p0 = x0[0:2]      # initial position
    unitr = np.array([[1, 0, 0, 0], [0, 1, 0, 0]], np.float32)
    in_maps = []
    for c in range(NCORES):
        # lhsT blocks [10, 128] each: weights replicated across partitions,
        # rows 8-9 = per-core constants (applied via the rhs unit rows)
        wmb = np.zeros((NCORES + 2, 4 * P), np.float64)
        for cp in range(c):
            wmb[cp, 0:P] = dt                                      # vel bias @ U
            wmb[cp, P:2 * P] = dt * dt                             # ramp coef @ U
            wmb[cp, 2 * P:3 * P] = dt * dt * (L * (c - cp) + 0.5)  # pos bias @ U
            wmb[cp, 3 * P:4 * P] = -dt * dt                        # pos bias @ J
        wmb[NCORES + 0, 0:P] = v0[0]
        wmb[NCORES + 1, 0:P] = v0[1]
        wmb[NCORES + 0, P:2 * P] = dt * v0[0]
        wmb[NCORES + 1, P:2 * P] = dt * v0[1]
        wmb[NCORES + 0, 2 * P:3 * P] = p0[0] + dt * c * L * v0[0]
        wmb[NCORES + 1, 2 * P:3 * P] = p0[1] + dt * c * L * v0[1]
        in_maps.append({
            "v": vchunks[c],
            "gtab": gtab,
            "taup": taups[c],
            "ltri": ltri,
            "ones": ones,
            "wmbig": np.ascontiguousarray(wmb.astype(np.float32)),
            "unitr": unitr,
        })
    return in_maps


def _host_seeds(tots, x0_mean):
    """Combine phase-A per-partition sums into per-partition scan initials.

    tots: list of NCORES arrays [128,4] = [sum u0, sum u1, sum j*u0, sum j*u1]
    Returns per-core [128,4] fp32: [initS_x, initS_y, initR_x, initR_y].
    """
    x0 = np.asarray(x0_mean, np.float64)
    seedS = x0[2:4] / DT            # running scan-1 state (u units)
    seedR = x0[0:2] / (DT * DT)     # running scan-2 state
    inits = []
    for c in range(NCORES):
        tot = np.asarray(tots[c], np.float64)
        U = tot[:, 0:2]             # per-partition sums of u
        J = tot[:, 2:4]             # per-partition sums of j*u (j = 1..F local)
        initS = np.empty((P, 2))
        initR = np.empty((P, 2))
        for p in range(P):
            initS[p] = seedS
            initR[p] = seedR
            flen = float(F if p < P - 1 else L - (P - 1) * F)  # true elements
            # sum over partition of scan-1 states = flen*seedS + sum (flen-j+1) u_j
            # (padded tail elements are zero so they contribute nothing)
            dR = flen * seedS + (flen + 1.0) * U[p] - J[p] - 0.5 * U[p]
            seedS = seedS + U[p]
            seedR = seedR + dR
        inits.append(np.ascontiguousarray(
            np.concatenate([initS, initR], axis=1).astype(np.float32)))
    return inits


def kernel(v_sequence, x0_mean, x0_cov, A, B, Q):
    v_sequence = np.asarray(v_sequence, np.float32)
    x0_mean = np.asarray(x0_mean, np.float32)
    x0_cov = np.asarray(x0_cov, np.float32)
    A = np.asarray(A, np.float32)
    B = np.asarray(B, np.float32)
    Q = np.asarray(Q, np.float32)

    core_ids = list(range(NCORES))
    import os
    if os.environ.get("KFUSED", "1") == "1":
        if "f" not in _CACHE:
            _CACHE["f"] = _build_fused()
        nc_f = _CACHE["f"]
        in_maps = _fused_in_maps(v_sequence, x0_mean, x0_cov, A, Q)
        out_m = run_bass_kernel_spmd(nc_f, in_maps, core_ids, trace=TRACE)
        res_m = out_m.results
        LAST_RESULTS.clear()
        LAST_RESULTS["m"] = out_m
    else:
        nc_a, nc_m = _get_kernels()
        vchunks, gtab, taups = _host_prep(v_sequence, x0_mean, x0_cov, A, Q)
        out_a = run_bass_kernel_spmd(
            nc_a, [{"v": vchunks[c]} for c in core_ids], core_ids, trace=TRACE)
        res_a = out_a.results
        inits = _host_seeds([r["tot"] for r in res_a], x0_mean)
        out_m = run_bass_kernel_spmd(
            nc_m,
            [{"v": vchunks[c], "gtab": gtab, "taup": taups[c], "init": inits[c]}
             for c in core_ids],
            core_ids, trace=TRACE)
        res_m = out_m.results
        LAST_RESULTS.clear()
        LAST_RESULTS["a"] = out_a
        LAST_RESULTS["m"] = out_m

    mean = np.empty((1, T + 1, 4), np.float32)
    cov = np.empty((1, T + 1, 4, 4), np.float32)
    mean[0, 0] = x0_mean
    cov[0, 0] = x0_cov
    for c in range(NCORES):
        mu = res_m[c]["mu_out"].reshape(PADCHUNK, 4)[:L]
        sg = res_m[c]["cov_out"].reshape(PADCHUNK, 16)[:L]
        mean[0, 1 + c * L:1 + (c + 1) * L] = mu
        cov[0, 1 + c * L:1 + (c + 1) * L] = sg.reshape(L, 4, 4)
    return mean, cov
